# revision 28
# baseline (speedup 1.0000x reference)
"""Trainium2 Bass kernel for the sliding-window-attention transformer
(nn_Model_22728966930624).

Sharding: sequence-parallel over 8 NeuronCores. Core c owns tokens
[c*512, (c+1)*512); each layer's K/V are computed over an extended region
with a 256-token halo on each side. Halos are refreshed between layers with
two staggered 8-rank AllGathers (fp8, right half first so the left halos —
needed first by attention block 0 — land early); both overlap the next
layer's own-token Q/K/V projections.

Precision: QKV/Wo/FFN1 weights and their activations run in fp8-e4m3 with
DoubleRow matmuls (weights prescaled x64 host-side; V additionally scaled
1/64 so the unnormalized fp8 attention accumulator stays inside TRN e4m3's
+-240 range — the reciprocal broadcast multiplies by 64/denominator to
compensate). FFN2 and the residual stream stay bf16: e4m3 noise through
GELU/LayerNorm nonlinearities rectifies into a systematic bias that costs
~1.4e-2 rel err if FFN2 is quantized. Softmax reciprocals and LayerNorm
rsqrt run on the scalar engine via raw-emitted Reciprocal/Rsqrt activations
(measured ~1e-5 accurate), grouped per attention block so the activation
table is not thrashed.

Device layout: activations are feature-major ("transposed") in SBUF:
x^T [768 rows -> 6 tiles x 128 partitions, tokens in the free dim].
LayerNorm statistics are computed with ones-matmuls on the PE; per-token
mean/rstd are broadcast across partitions with K=1 matmuls into PSUM.
Softmax is computed max-free (scores are O(1) by construction) with a
multiplicative band mask, and the softmax denominator comes for free from
an extra ones-column appended to each attention head's V block. The
attention head loop is software-pipelined (scores of head h+1 are emitted
before the PV matmuls of head h) so the PE never waits on exp/mask.
"""
import os
import sys
import types

import numpy as np
import ml_dtypes

import concourse.bass as bass
import concourse.mybir as mybir
import concourse.tile as tile
from concourse.alu_op_type import AluOpType
from concourse.bass_utils import run_bass_kernel_spmd

F32 = mybir.dt.float32
BF16 = mybir.dt.bfloat16
FP8 = mybir.dt.float8e4
FP8_WSCALE = 64.0
AF = mybir.ActivationFunctionType
NPBF16 = ml_dtypes.bfloat16

# model dims
S, D, H, DH, L, FF = 4096, 768, 12, 64, 4, 3072
C, W = 256, 256
P = 8                   # cores
T_OWN = S // P          # 512
T_EXT = T_OWN + 2 * C   # 1024
NJ = D // 128           # 6 feature row-tiles
NJF = FF // 128         # 24
HS = DH + 1             # 65: V head slot width (extra ones column)
HSP = 68                # fp8 V slot padded so tt-pair stride is 16B-aligned

# bias/gamma column registry (shared host/device)
PER_LAYER_COLS = 72
NB = 12 + L * PER_LAYER_COLS


def col_emb_g(j): return j
def col_emb_b(j): return 6 + j
def lbase(l): return 12 + l * PER_LAYER_COLS
def col_bq(l, j): return lbase(l) + j
def col_bk(l, j): return lbase(l) + 6 + j
def col_bo(l, j): return lbase(l) + 12 + j
def col_bff2(l, j): return lbase(l) + 18 + j
def col_bff1(l, j): return lbase(l) + 24 + j       # j in 0..23
def col_ln1g(l, j): return lbase(l) + 48 + j
def col_ln1b(l, j): return lbase(l) + 54 + j
def col_ln2g(l, j): return lbase(l) + 60 + j
def col_ln2b(l, j): return lbase(l) + 66 + j


_MAX_WAITS = 1


def _split_excess_waits(nc, max_waits=_MAX_WAITS):
    """This walrus build rejects >1 semaphore wait per instruction; move
    extras onto same-engine NoOps inserted just before."""
    n = 0
    for f in nc.m.functions:
        for bb in f.blocks:
            new_insts = []
            for inst in bb.instructions:
                si = inst.sync_info
                if si is not None and si.on_wait and len(si.on_wait) > max_waits:
                    excess = list(si.on_wait[:-max_waits])
                    keep = list(si.on_wait[-max_waits:])
                    for k, w in enumerate(excess):
                        nop = mybir.InstNoOp(name=f"{inst.name}-wsplit{k}")
                        nop.engine = inst.engine
                        nop.sync_info = mybir.SyncInfo(on_wait=[w], on_update=[])
                        new_insts.append(nop)
                        n += 1
                    inst.sync_info = mybir.SyncInfo(
                        on_wait=keep, on_update=list(si.on_update)
                    )
                new_insts.append(inst)
            bb.instructions[:] = new_insts
    return n


def _act_raw(nc, out, in_, func, bias=0.0, scale=1.0):
    """nc.scalar.activation minus the Reciprocal/Rsqrt accuracy guard."""
    eng = nc.scalar
    inputs = [eng.lower_ap(in_)]
    for arg in (bias, scale, 0.0):
        if isinstance(arg, bass.AP):
            inputs.append(eng.lower_ap(arg))
        else:
            inputs.append(mybir.ImmediateValue(dtype=mybir.dt.float32, value=arg))
    return eng.add_instruction(
        mybir.InstActivation(
            name=nc.get_next_instruction_name(),
            func=func, ins=inputs, outs=[eng.lower_ap(out)]))


def _install_ntff_hook():
    if "antenv.axon_hooks" in sys.modules:
        return
    try:
        from trn_agent_boot.trn_boot import _ntff_profile_via_ctypes
        hook = _ntff_profile_via_ctypes("/opt/axon/libaxon_pjrt.so")
    except Exception:
        hook = None
    mod = types.ModuleType("antenv.axon_hooks")
    mod.get_axon_ntff_profile_hook = lambda: hook
    mod.set_axon_ntff_profile_hook = lambda h: None
    sys.modules["antenv.axon_hooks"] = mod
    try:
        import antenv
        antenv.axon_hooks = mod
    except Exception:
        pass


# --------------------------------------------------------------------------
# device program
# --------------------------------------------------------------------------

def build_program(n_layers=L, need_mid=False):  # noqa: C901
    nc = bass.Bass("TRN2", target_bir_lowering=False, debug=False,
                   enable_asserts=True, num_devices=P)
    io = {}
    io["embT"] = nc.dram_tensor("embT", [D, T_EXT], BF16, kind="ExternalInput").ap()
    for nm in ("wq", "wk", "wv"):
        io[nm] = nc.dram_tensor(nm, [128, L, NJ, D], FP8, kind="ExternalInput").ap()
    io["wo"] = nc.dram_tensor("wo", [128, L, NJ, D], FP8, kind="ExternalInput").ap()
    io["wf1"] = nc.dram_tensor("wf1", [128, L, 4, NJ, D], FP8, kind="ExternalInput").ap()
    io["wf2"] = nc.dram_tensor("wf2", [128, L, 4, NJ, D], BF16, kind="ExternalInput").ap()
    io["bias_cols"] = nc.dram_tensor("bias_cols", [128, NB], F32, kind="ExternalInput").ap()
    io["bv_rows"] = nc.dram_tensor("bv_rows", [1, L * D], BF16, kind="ExternalInput").ap()
    nmt = 3 if need_mid else 2
    io["maskT"] = nc.dram_tensor("maskT", [128, 2, nmt, 512], BF16, kind="ExternalInput").ap()
    io["maskf"] = nc.dram_tensor("maskf", [1, T_OWN], F32, kind="ExternalInput").ap()
    io["pool_out"] = nc.dram_tensor("pool_out", [128, NJ], F32, kind="ExternalOutput").ap()

    with tile.TileContext(nc) as tc:
        _build_tile_kernel(tc, io, n_layers, need_mid)
    _split_excess_waits(nc)
    return nc


def _build_tile_kernel(tc, io, n_layers, need_mid):
    nc = tc.nc
    from contextlib import ExitStack

    ctx = ExitStack()
    with ctx:
        consts = ctx.enter_context(tc.tile_pool(name="consts", bufs=1))
        xn_pool = ctx.enter_context(tc.tile_pool(name="xn", bufs=1))
        r_pool = ctx.enter_context(tc.tile_pool(name="rp", bufs=2))
        xb_pool = ctx.enter_context(tc.tile_pool(name="xb", bufs=1))
        kqa_pool = ctx.enter_context(tc.tile_pool(name="kqa", bufs=1))
        v_pool = ctx.enter_context(tc.tile_pool(name="vp", bufs=1))
        h_pool = ctx.enter_context(tc.tile_pool(name="hp", bufs=1))
        w_pool = ctx.enter_context(tc.tile_pool(name="wp", bufs=3))
        wf1_pool = ctx.enter_context(tc.tile_pool(name="wf1p", bufs=3))
        wf2_pool = ctx.enter_context(tc.tile_pool(name="wf2p", bufs=2))
        den_pool = ctx.enter_context(tc.tile_pool(name="denp", bufs=2))
        exp_pool = ctx.enter_context(tc.tile_pool(name="expp", bufs=5))
        tmp_pool = ctx.enter_context(tc.tile_pool(name="tmpp", bufs=4))
        sq_pool = ctx.enter_context(tc.tile_pool(name="sqp", bufs=3))
        vec_pool = ctx.enter_context(tc.tile_pool(name="vecp", bufs=3))
        acc_pool = ctx.enter_context(tc.tile_pool(name="accp", bufs=1))
        dram_pool = ctx.enter_context(tc.tile_pool(name="dram", bufs=2, space="DRAM"))
        ps_pool = ctx.enter_context(tc.tile_pool(name="bigps", bufs=6, space="PSUM"))
        attn_ps = ctx.enter_context(tc.tile_pool(name="attnps", bufs=2, space="PSUM"))

        # ---- constants ----
        ones_col = consts.tile([128, 1], F32)
        nc.vector.memset(ones_col, 1.0)
        ones_row = consts.tile([1, 128], F32)
        nc.vector.memset(ones_row, 1.0)
        ones_row_bf = consts.tile([1, 128], BF16)
        nc.vector.memset(ones_row_bf, 1.0)
        ones_col_bf = consts.tile([128, 1], BF16)
        nc.vector.memset(ones_col_bf, 1.0)
        row64 = consts.tile([1, 128], F32)
        nc.vector.memset(row64, 8.0)
        bias_sb = consts.tile([128, NB], F32)
        nc.sync.dma_start(out=bias_sb, in_=io["bias_cols"])
        bv_sb = consts.tile([1, L * D], BF16)
        nc.sync.dma_start(out=bv_sb, in_=io["bv_rows"])
        mask_sb = consts.tile([128, 2, 3 if need_mid else 2, 512], BF16)
        nc.sync.dma_start(out=mask_sb, in_=io["maskT"])
        mslot = {0: 0, 1: 1, 2: 2} if need_mid else {0: 0, 2: 1}
        maskf_sb = consts.tile([1, T_OWN], F32)
        nc.sync.dma_start(out=maskf_sb, in_=io["maskf"])
        eps_col = consts.tile([1, 1], F32)
        nc.vector.memset(eps_col, 1e-5)

        def bcol(idx):
            return bias_sb[:, idx:idx + 1]

        pid = nc.partition_id()
        lidx6 = ((pid + P - 1) % P) * NJ
        ridx6 = ((pid + 1) % P) * NJ

        # ---------------- layer norm helper ----------------
        def layer_norm(src_stats, src_apply, nblk, g_col, b_col, outs,
                       ones_st=None, bw=512, blk_done=None):
            """src_*(j, blk) -> AP f32/bf16 [128,bw].
            outs(j, blk) -> list of (dst_ap, lo, hi): dst = g*t2[:, lo:hi] + b."""
            for blk in range(nblk):
                sum_ps = attn_ps.tile([1, bw], F32, tag="attn")
                sq_ps = attn_ps.tile([1, bw], F32, tag="attn")
                for j in range(NJ):
                    s = src_stats(j, blk)
                    sq = sq_pool.tile([128, 512], F32, tag="sq", bufs=2)
                    sq = sq[:, 0:bw]
                    nc.vector.tensor_tensor(sq, s, s, AluOpType.mult)
                    nc.tensor.matmul(sum_ps, ones_st or ones_col, s,
                                     start=(j == 0), stop=(j == NJ - 1))
                    nc.tensor.matmul(sq_ps, ones_col, sq,
                                     start=(j == 0), stop=(j == NJ - 1))
                mean = vec_pool.tile([1, 512], F32, tag="vec", name="mean")[:, 0:bw]
                nc.vector.tensor_scalar(mean, sum_ps, 1.0 / D, None, AluOpType.mult)
                ex2 = vec_pool.tile([1, 512], F32, tag="vec", name="ex2")[:, 0:bw]
                nc.vector.tensor_scalar(ex2, sq_ps, 1.0 / D, None, AluOpType.mult)
                var = vec_pool.tile([1, 512], F32, tag="vec", name="var")[:, 0:bw]
                nc.vector.tensor_tensor(var, mean, mean, AluOpType.mult)
                nc.vector.tensor_tensor(var, ex2, var, AluOpType.subtract)
                rstd = vec_pool.tile([1, 512], F32, tag="vec", name="rstd")[:, 0:bw]
                _act_raw(nc, rstd, var, AF.Rsqrt, bias=eps_col)
                mb = ps_pool.tile([128, 512], F32, tag="big", name="mb")[:, 0:bw]
                nc.tensor.matmul(mb, ones_row, mean, start=True, stop=True)
                rb = ps_pool.tile([128, 512], F32, tag="big", name="rb")[:, 0:bw]
                nc.tensor.matmul(rb, ones_row, rstd, start=True, stop=True)
                mb_sb = tmp_pool.tile([128, 512], F32, tag="tmp", name="mb_sb")[:, 0:bw]
                nc.vector.tensor_copy(mb_sb, mb)
                rb_sb = tmp_pool.tile([128, 512], F32, tag="tmp", name="rb_sb")[:, 0:bw]
                nc.vector.tensor_copy(rb_sb, rb)
                for j in range(NJ):
                    s = src_apply(j, blk)
                    t1 = tmp_pool.tile([128, 512], F32, tag="tmp", name="t1")[:, 0:bw]
                    nc.vector.tensor_tensor(t1, s, mb_sb, AluOpType.subtract)
                    t2 = tmp_pool.tile([128, 512], F32, tag="tmp", name="t2")[:, 0:bw]
                    nc.vector.tensor_tensor(t2, t1, rb_sb, AluOpType.mult)
                    for dst, lo, hi in outs(j, blk):
                        nc.vector.tensor_scalar(dst, t2[:, lo:hi],
                                                bcol(g_col(j)), bcol(b_col(j)),
                                                AluOpType.mult, AluOpType.add)
                if blk_done is not None:
                    blk_done(blk)

        # ---------------- embedding layer norm (over ext tokens) ----------
        xn = xn_pool.tile([128, NJ, T_EXT], FP8, tag="xn")
        r0 = r_pool.tile([128, NJ, T_OWN], BF16, tag="r")

        def emb_src(which):
            def get(j, blk):
                t = exp_pool.tile([128, 512], BF16, tag="exp", bufs=5, name="embt")
                nc.sync.dma_start(
                    out=t,
                    in_=io["embT"][j * 128:(j + 1) * 128, blk * 512:(blk + 1) * 512])
                return t
            return get

        def emb_outs(j, blk):
            dsts = [(xn[:, j, blk * 512:(blk + 1) * 512], 0, 512)]
            if blk == 0:
                dsts.append((r0[:, j, 0:256], 256, 512))
            else:
                dsts.append((r0[:, j, 256:512], 0, 256))
            return dsts

        with nc.named_scope("emb_ln"):
            layer_norm(emb_src(0), emb_src(1), 2, col_emb_g, col_emb_b, emb_outs,
                       ones_st=ones_col_bf)

        # ---------------- transformer layers ----------------
        def load_qkvo(l):
            ws = []
            for nm in ("wq", "wk", "wv", "wo"):
                w = w_pool.tile([128, NJ, D], FP8, tag="w8", bufs=8,
                                name=f"{nm}{l}")
                nc.sync.dma_start(out=w, in_=io[nm][:, l])
                ws.append(w)
            return ws

        wtiles = load_qkvo(0)
        for l in range(n_layers):
            first = (l == 0)
            last = (l == n_layers - 1)

            wq_sb, wk_sb, wv_sb, wo_sb = wtiles

            kT = kqa_pool.tile([128, NJ, T_EXT], BF16, tag="kT")
            qT = kqa_pool.tile([128, NJ, T_OWN], BF16, tag="qT")
            v_sb = v_pool.tile([128, 8, H, HSP], FP8, tag="v")
            nc.vector.memset(v_sb[:, :, :, DH:HS], 1.0)

            def k_proj(tlo, thi):
                nn = thi - tlo
                for mj in range(NJ):
                    ps = ps_pool.tile([128, 512], F32, tag="big")
                    for kj in range(0, NJ, 2):
                        nc.tensor.matmul(
                            ps[:, 0:nn], wk_sb[:, kj:kj + 2, mj * 128:(mj + 1) * 128],
                            xn[:, kj:kj + 2, tlo:thi],
                            start=(kj == 0), stop=(kj == NJ - 2),
                            perf_mode=mybir.MatmulPerfMode.DoubleRow)
                    nc.vector.tensor_scalar(kT[:, mj, tlo:thi], ps[:, 0:nn],
                                            1.0 / FP8_WSCALE, bcol(col_bk(l, mj)),
                                            AluOpType.mult, AluOpType.add)

            def v_proj(tts):
                for tt in tts:
                    for ob in range(2):
                        psfull = ps_pool.tile([128, 512], F32, tag="big")
                        ps = psfull[:, 0:384]
                        for kj in range(0, NJ, 2):
                            nc.tensor.matmul(
                                ps, xn[:, kj:kj + 2, tt * 128:(tt + 1) * 128],
                                wv_sb[:, kj:kj + 2, ob * 384:(ob + 1) * 384],
                                start=(kj == 0), stop=False,
                                perf_mode=mybir.MatmulPerfMode.DoubleRow)
                        nc.tensor.matmul(
                            ps, ones_row_bf,
                            bv_sb[:, l * D + ob * 384: l * D + (ob + 1) * 384],
                            start=False, stop=True)
                        nc.vector.tensor_scalar(
                            v_sb[:, tt, ob * 6:(ob + 1) * 6, 0:DH],
                            ps.rearrange("p (h s) -> p h s", s=DH),
                            1.0 / (FP8_WSCALE * 8.0), None, AluOpType.mult)

            # -- own-token projections (independent of the halo AllGather) --
            with nc.named_scope(f"L{l}.qkv_own"):
                for mj in range(NJ):
                    ps = ps_pool.tile([128, 512], F32, tag="big")
                    for kj in range(0, NJ, 2):
                        nc.tensor.matmul(
                            ps, wq_sb[:, kj:kj + 2, mj * 128:(mj + 1) * 128],
                            xn[:, kj:kj + 2, 256:768],
                            start=(kj == 0), stop=(kj == NJ - 2),
                            perf_mode=mybir.MatmulPerfMode.DoubleRow)
                    nc.vector.tensor_scalar(qT[:, mj, :], ps,
                                            1.0 / FP8_WSCALE, bcol(col_bq(l, mj)),
                                            AluOpType.mult, AluOpType.add)
                if first:
                    k_proj(0, 512)
                    k_proj(512, 1024)
                    v_proj(range(8))
                else:
                    k_proj(256, 768)
                    v_proj(range(2, 6))

            # -- attention (heads software-pipelined) --
            attnT = kqa_pool.tile([128, NJ, T_OWN], FP8, tag="attnT")

            def emit_scores(n, h):
                jh, po = h // 2, (h % 2) * 64
                ems = []
                for t in range(3):
                    ps = ps_pool.tile([128, 512], F32, tag="big")
                    for half in range(2):
                        kofs = n * 256 + (2 * t + half) * 128
                        nc.tensor.matmul(
                            ps[:, half * 256:(half + 1) * 256],
                            kT[po:po + 64, jh, kofs:kofs + 128],
                            qT[po:po + 64, jh, n * 256:(n + 1) * 256],
                            start=True, stop=True)
                    e = exp_pool.tile([128, 512], FP8, tag="exp")
                    nc.scalar.activation(e, ps, AF.Exp)
                    if t == 1 and not need_mid:
                        ems.append(e)
                    else:
                        em = exp_pool.tile([128, 512], FP8, tag="em")
                        nc.vector.tensor_tensor(
                            em, e, mask_sb[:, n, mslot[t], :], AluOpType.mult)
                        ems.append(em)
                return ems

            dens = [den_pool.tile([1, H * 256], BF16, tag="den", name=f"den{i}")
                    for i in range(2)]

            def emit_av(n, h, ems):
                jh, po = h // 2, (h % 2) * 64
                aps = attn_ps.tile([HS, 256], F32, tag="attn")
                for t in range(3):
                    tt = n * 2 + 2 * t
                    nc.tensor.matmul(
                        aps, v_sb[:, tt:tt + 2, h, 0:HS],
                        ems[t].rearrange("p (k q) -> p k q", k=2),
                        start=(t == 0), stop=(t == 2),
                        perf_mode=mybir.MatmulPerfMode.DoubleRow)
                nc.vector.tensor_copy(dens[n][:, h * 256:(h + 1) * 256],
                                      aps[64:65, :])
                nc.vector.tensor_copy(
                    attnT[po:po + 64, jh, n * 256:(n + 1) * 256], aps[0:64, :])

            def emit_norm(n):
                # grouped reciprocals: one ACT table load for the whole block
                for h in range(H):
                    jh, po = h // 2, (h % 2) * 64
                    rec = vec_pool.tile([1, 256], F32, tag="rec")
                    _act_raw(nc, rec, dens[n][:, h * 256:(h + 1) * 256],
                             AF.Reciprocal)
                    bc = attn_ps.tile([64, 256], F32, tag="attn")
                    nc.tensor.matmul(bc, row64[0:1, 0:64], rec,
                                     start=True, stop=True)
                    sl = attnT[po:po + 64, jh, n * 256:(n + 1) * 256]
                    nc.vector.tensor_tensor(sl, sl, bc, AluOpType.mult)

            pend = None
            for n in range(2):
                if n == 1:
                    if not first:
                        with nc.named_scope(f"L{l}.halo_right"):
                            k_proj(768, 1024)
                            v_proj(range(6, 8))
                    if not last:
                        wtiles = load_qkvo(l + 1)
                with nc.named_scope(f"L{l}.attn{n}"):
                    if n == 0 and not first:
                        with nc.named_scope(f"L{l}.halo_left"):
                            k_proj(0, 256)
                            v_proj(range(0, 2))
                    for h in range(H):
                        ems = emit_scores(n, h)
                        if pend is not None:
                            emit_av(*pend)
                        pend = (n, h, ems)
                        if n == 1 and h == 1:
                            emit_norm(0)
            emit_av(*pend)
            emit_norm(1)

            # -- Wo projection + residual -> r1 --
            r1 = r_pool.tile([128, NJ, T_OWN], BF16, tag="r")
            with nc.named_scope(f"L{l}.wo"):
                for mj in range(NJ):
                    ps = ps_pool.tile([128, 512], F32, tag="big")
                    for kj in range(0, NJ, 2):
                        nc.tensor.matmul(
                            ps, wo_sb[:, kj:kj + 2, mj * 128:(mj + 1) * 128],
                            attnT[:, kj:kj + 2, :],
                            start=(kj == 0), stop=(kj == NJ - 2),
                            perf_mode=mybir.MatmulPerfMode.DoubleRow)
                    t = sq_pool.tile([128, 512], F32, tag="ao", bufs=2)
                    nc.vector.tensor_scalar(t, ps, 1.0 / FP8_WSCALE,
                                            bcol(col_bo(l, mj)),
                                            AluOpType.mult, AluOpType.add)
                    nc.vector.tensor_tensor(r1[:, mj, :], t, r0[:, mj, :],
                                            AluOpType.add)

            # -- LN1 -> xn1b (bf16; also the FFN residual) --
            xn1b = xb_pool.tile([128, NJ, T_OWN], BF16, tag="xn1b")
            xn1b8 = xb_pool.tile([128, NJ, T_OWN], FP8, tag="xn1b8")
            with nc.named_scope(f"L{l}.ln1"):
                layer_norm(
                    lambda j, blk: r1[:, j, :], lambda j, blk: r1[:, j, :], 1,
                    lambda j: col_ln1g(l, j), lambda j: col_ln1b(l, j),
                    lambda j, blk: [(xn1b[:, j, :], 0, 512),
                                    (xn1b8[:, j, :], 0, 512)],
                    ones_st=ones_col_bf)

            # -- FFN --
            r2 = r_pool.tile([128, NJ, T_OWN], BF16, tag="r")
            with nc.named_scope(f"L{l}.ffn"):
                f2ps = [ps_pool.tile([128, 512], F32, tag="big",
                                     name=f"f2ps{mj}") for mj in range(NJ)]
                for q in range(4):
                    wf1_sb = wf1_pool.tile([128, NJ, D], FP8, tag="wf1")
                    nc.gpsimd.dma_start(out=wf1_sb, in_=io["wf1"][:, l, q])
                    wf2_sb = wf2_pool.tile([128, NJ, D], BF16, tag="wf2")
                    nc.gpsimd.dma_start(out=wf2_sb, in_=io["wf2"][:, l, q])
                    hq = h_pool.tile([128, NJ, T_OWN], BF16, tag="h", bufs=2)
                    for mj6 in range(NJ):
                        ps = attn_ps.tile([128, 512], F32, tag="attn")
                        for kj in range(0, NJ, 2):
                            nc.tensor.matmul(
                                ps, wf1_sb[:, kj:kj + 2, mj6 * 128:(mj6 + 1) * 128],
                                xn1b8[:, kj:kj + 2, :],
                                start=(kj == 0), stop=(kj == NJ - 2),
                                perf_mode=mybir.MatmulPerfMode.DoubleRow)
                        nc.scalar.activation(
                            hq[:, mj6, :], ps, AF.Gelu,
                            bias=bcol(col_bff1(l, q * NJ + mj6)),
                            scale=1.0 / FP8_WSCALE)
                    for mj in range(NJ):
                        for kj in range(NJ):
                            nc.tensor.matmul(
                                f2ps[mj], wf2_sb[:, kj, mj * 128:(mj + 1) * 128],
                                hq[:, kj, :],
                                start=(q == 0 and kj == 0),
                                stop=(q == 3 and kj == NJ - 1))
                for mj in range(NJ):
                    t = sq_pool.tile([128, 512], F32, tag="ao", bufs=2)
                    nc.vector.tensor_scalar(t, f2ps[mj], bcol(col_bff2(l, mj)),
                                            None, AluOpType.add)
                    nc.vector.tensor_tensor(r2[:, mj, :], t, xn1b[:, mj, :],
                                            AluOpType.add)

            # -- LN2 -> next xn (+ next r0), in halves; right half first so
            # its AllGather (which feeds the left halos) launches early --
            xn_next = None if last else xn_pool.tile([128, NJ, T_EXT], FP8, tag="xn")
            xn2f = r_pool.tile([128, NJ, T_OWN], BF16, tag="r")
            HLO = (256, 0)  # blk0 = own tokens 256:512, blk1 = 0:256

            def ln2_outs(j, blk, xn_next=xn_next, xn2f=xn2f, last=last):
                lo = HLO[blk]
                dsts = [(xn2f[:, j, lo:lo + 256], 0, 256)]
                if not last:
                    dsts.append((xn_next[:, j, 256 + lo:512 + lo], 0, 256))
                return dsts

            def ln2_blk_done(blk, xn_next=xn_next, last=last, l=l):
                if last:
                    return
                with nc.named_scope(f"L{l}.allgather{blk}"):
                    agi = dram_pool.tile([D, 256], FP8, tag="agi")
                    ago = dram_pool.tile([P * D, 256], FP8, tag="ago",
                                         addr_space="Shared")
                    lo = HLO[blk]
                    nc.sync.dma_start(
                        out=agi.rearrange("(j p) t -> p j t", p=128),
                        in_=xn_next[:, :, 256 + lo:512 + lo])
                    nc.gpsimd.collective_compute(
                        "AllGather", AluOpType.bypass,
                        replica_groups=[list(range(P))],
                        ins=[agi.opt()], outs=[ago.opt()])
                    agv = ago.rearrange("(r j p) t -> p (r j) t", j=NJ, p=128)
                    if blk == 0:
                        nc.sync.dma_start(out=xn_next[:, :, 0:256],
                                          in_=agv[:, bass.ds(lidx6, NJ), :])
                    else:
                        nc.sync.dma_start(out=xn_next[:, :, 768:1024],
                                          in_=agv[:, bass.ds(ridx6, NJ), :])

            with nc.named_scope(f"L{l}.ln2"):
                layer_norm(
                    lambda j, blk, r2=r2: r2[:, j, HLO[blk]:HLO[blk] + 256],
                    lambda j, blk, r2=r2: r2[:, j, HLO[blk]:HLO[blk] + 256], 2,
                    lambda j: col_ln2g(l, j), lambda j: col_ln2b(l, j),
                    ln2_outs, ones_st=ones_col_bf, bw=256,
                    blk_done=ln2_blk_done)

            if not last:
                xn = xn_next
            r0 = xn2f

        # ---------------- pooling partials ----------------
        with nc.named_scope("pool"):
            accs = acc_pool.tile([128, NJ], F32, tag="accs")
            if need_mid:
                mb = ps_pool.tile([128, 512], F32, tag="big")
                nc.tensor.matmul(mb, ones_row, maskf_sb, start=True, stop=True)
                for j in range(NJ):
                    mskd = tmp_pool.tile([128, 512], F32, tag="tmp")
                    nc.vector.tensor_tensor(mskd, r0[:, j, :], mb, AluOpType.mult)
                    scr = sq_pool.tile([128, 512], F32, tag="sq", bufs=2)
                    nc.scalar.activation(scr, mskd, AF.Copy,
                                         accum_out=accs[:, j:j + 1])
            else:
                for j in range(NJ):
                    scr = sq_pool.tile([128, 512], F32, tag="sq", bufs=2)
                    nc.scalar.activation(scr, r0[:, j, :], AF.Copy,
                                         accum_out=accs[:, j:j + 1])
            nc.sync.dma_start(out=io["pool_out"], in_=accs)


# --------------------------------------------------------------------------
# host side
# --------------------------------------------------------------------------

def _build_masks(attention_mask):
    """[P, 2, 3*C, C] multiplicative float mask (band + attn mask + edges)."""
    maskf = np.asarray(attention_mask, np.float32).reshape(S)
    masks = np.zeros((P, 2, 3 * C, C), np.float32)
    qi = np.arange(C)[None, :]
    kj = np.arange(3 * C)[:, None]
    band = (np.abs(kj - C - qi) <= W)
    for c in range(P):
        for n in range(2):
            g0 = c * T_OWN + n * C
            kg = g0 - C + np.arange(3 * C)
            valid = (kg >= 0) & (kg < S)
            mvals = np.where(valid, maskf[np.clip(kg, 0, S - 1)], 0.0)
            masks[c, n] = band * (mvals[:, None] > 0)
    return masks


_cache = {}


def kernel(input_ids, attention_mask, word_emb, pos_emb, emb_g, emb_b,
           Wq, Wk, Wv, Wo, bq, bk, bv, bo, ln1_g, ln1_b,
           Wff1, bff1, Wff2, bff2, ln2_g, ln2_b,
           W1, b1, W2, b2, W3, b3):
    to32 = lambda a: np.ascontiguousarray(np.asarray(a, np.float32))
    tob = lambda a: np.asarray(a, np.float32).astype(NPBF16)
    to8 = lambda a: np.clip(np.asarray(a, np.float32) * FP8_WSCALE,
                            -240, 240).astype(ml_dtypes.float8_e4m3)
    ids = np.asarray(input_ids).reshape(S)
    word_emb, pos_emb = to32(word_emb), to32(pos_emb)
    emb = word_emb[ids] + pos_emb                      # [S, D] host gather
    masks = _build_masks(attention_mask)
    maskf = np.asarray(attention_mask, np.float32).reshape(S)
    need_mid = not bool(np.asarray(attention_mask).all())

    scale = 1.0 / np.sqrt(np.float32(DH))
    wq_s = to32(Wq) * scale
    bq_s = to32(bq) * scale

    bias_cols = np.zeros((128, NB), np.float32)
    for j in range(NJ):
        sl = slice(j * 128, (j + 1) * 128)
        bias_cols[:, col_emb_g(j)] = to32(emb_g)[sl]
        bias_cols[:, col_emb_b(j)] = to32(emb_b)[sl]
    for l in range(L):
        for j in range(NJ):
            sl = slice(j * 128, (j + 1) * 128)
            bias_cols[:, col_bq(l, j)] = bq_s[l][sl]
            bias_cols[:, col_bk(l, j)] = to32(bk)[l][sl]
            bias_cols[:, col_bo(l, j)] = to32(bo)[l][sl]
            bias_cols[:, col_bff2(l, j)] = to32(bff2)[l][sl]
            bias_cols[:, col_ln1g(l, j)] = to32(ln1_g)[l][sl]
            bias_cols[:, col_ln1b(l, j)] = to32(ln1_b)[l][sl]
            bias_cols[:, col_ln2g(l, j)] = to32(ln2_g)[l][sl]
            bias_cols[:, col_ln2b(l, j)] = to32(ln2_b)[l][sl]
        for j in range(NJF):
            bias_cols[:, col_bff1(l, j)] = to32(bff1)[l][j * 128:(j + 1) * 128]

    # weights pre-transposed host-side to [128, L, (q,) kj, out] so each
    # per-layer DMA reads one contiguous run per partition
    wq_b = np.ascontiguousarray(
        to8(wq_s).reshape(L, NJ, 128, D).transpose(2, 0, 1, 3))
    wk_b = np.ascontiguousarray(
        to8(Wk).reshape(L, NJ, 128, D).transpose(2, 0, 1, 3))
    wv_b = np.ascontiguousarray(
        to8(Wv).reshape(L, NJ, 128, D).transpose(2, 0, 1, 3))
    wo_b = np.ascontiguousarray(
        to8(Wo).reshape(L, NJ, 128, D).transpose(2, 0, 1, 3))
    wf1_b = np.ascontiguousarray(
        to8(Wff1).reshape(L, NJ, 128, 4, D).transpose(2, 0, 3, 1, 4))
    wf2_b = np.ascontiguousarray(
        tob(Wff2).reshape(L, 4, NJ, 128, D).transpose(3, 0, 1, 2, 4))
    bv_b = np.ascontiguousarray(tob(np.asarray(bv, np.float32)
                                    * FP8_WSCALE).reshape(1, L * D))

    n_layers = int(os.environ.get("KERNEL_LAYERS", L))
    key = (n_layers, need_mid)
    if key not in _cache:
        _cache[key] = build_program(n_layers, need_mid)
    nc = _cache[key]

    in_maps = []
    for c in range(P):
        lo, hi = c * T_OWN - C, c * T_OWN + T_OWN + C
        e = np.zeros((T_EXT, D), np.float32)
        s0, s1 = max(lo, 0), min(hi, S)
        e[s0 - lo:s1 - lo] = emb[s0:s1]
        mp = np.zeros((2, 3, 128, 512), np.float32)
        for n in range(2):
            for t in range(3):
                for half in range(2):
                    mp[n, t, :, half * 256:(half + 1) * 256] = \
                        masks[c, n][(2 * t + half) * 128:(2 * t + half + 1) * 128, :]
        in_maps.append({
            "embT": np.ascontiguousarray(e.T.astype(NPBF16)),
            "wq": wq_b, "wk": wk_b, "wv": wv_b, "wo": wo_b,
            "wf1": wf1_b, "wf2": wf2_b,
            "bias_cols": bias_cols,
            "bv_rows": bv_b,
            "maskT": np.ascontiguousarray(
                mp.transpose(2, 0, 1, 3)[:, :, ([0, 1, 2] if need_mid else [0, 2])]
                .astype(NPBF16)),
            "maskf": np.ascontiguousarray(
                maskf[c * T_OWN:(c + 1) * T_OWN].reshape(1, T_OWN)),
        })

    trace = os.environ.get("KERNEL_TRACE", "0") == "1"
    if trace:
        _install_ntff_hook()
    res = run_bass_kernel_spmd(nc, in_maps, core_ids=list(range(P)), trace=trace)
    kernel.last_exec_time_ns = res.exec_time_ns
    kernel.last_results = res.results
    kernel.last_res = res

    pooled = np.zeros(D, np.float64)
    for c in range(P):
        po = np.asarray(res.results[c]["pool_out"], np.float64)   # [128, NJ]
        pooled += po.T.reshape(D)                                 # f = j*128+p
    msum = max(maskf.sum(), 1e-9)
    pooled = (pooled / msum).astype(np.float32)

    h1 = np.maximum(pooled @ to32(W1) + to32(b1), 0)
    h2 = np.maximum(h1 @ to32(W2) + to32(b2), 0)
    pred = (h2 @ to32(W3) + to32(b3))[None].astype(np.float32)
    return pred, pred


kernel.last_exec_time_ns = None
kernel.last_results = None
kernel.last_res = None


# revision 29
# speedup vs baseline: 1.0640x; 1.0640x over previous
"""Trainium2 Bass kernel for the sliding-window-attention transformer
(nn_Model_22728966930624).

Sharding: sequence-parallel over 8 NeuronCores. Core c owns tokens
[c*512, (c+1)*512); each layer's K/V are computed over an extended region
with a 256-token halo on each side. Halos are refreshed between layers with
two staggered 8-rank AllGathers (fp8, right half first so the left halos —
needed first by attention block 0 — land early); both overlap the next
layer's own-token Q/K/V projections.

Precision: QKV/Wo/FFN1 weights and their activations run in fp8-e4m3 with
DoubleRow matmuls (weights prescaled x64 host-side; V additionally scaled
1/64 so the unnormalized fp8 attention accumulator stays inside TRN e4m3's
+-240 range — the reciprocal broadcast multiplies by 64/denominator to
compensate). FFN2 and the residual stream stay bf16: e4m3 noise through
GELU/LayerNorm nonlinearities rectifies into a systematic bias that costs
~1.4e-2 rel err if FFN2 is quantized. Softmax reciprocals and LayerNorm
rsqrt run on the scalar engine via raw-emitted Reciprocal/Rsqrt activations
(measured ~1e-5 accurate), grouped per attention block so the activation
table is not thrashed.

Device layout: activations are feature-major ("transposed") in SBUF:
x^T [768 rows -> 6 tiles x 128 partitions, tokens in the free dim].
LayerNorm statistics are computed with ones-matmuls on the PE; per-token
mean/rstd are broadcast across partitions with K=1 matmuls into PSUM.
Softmax is computed max-free (scores are O(1) by construction) with a
multiplicative band mask, and the softmax denominator comes for free from
an extra ones-column appended to each attention head's V block. The
attention head loop is software-pipelined (scores of head h+1 are emitted
before the PV matmuls of head h) so the PE never waits on exp/mask.
"""
import os
import sys
import types

import numpy as np
import ml_dtypes

import concourse.bass as bass
import concourse.mybir as mybir
import concourse.tile as tile
from concourse.alu_op_type import AluOpType
from concourse.bass_utils import run_bass_kernel_spmd

F32 = mybir.dt.float32
BF16 = mybir.dt.bfloat16
FP8 = mybir.dt.float8e4
FP8_WSCALE = 64.0
AF = mybir.ActivationFunctionType
NPBF16 = ml_dtypes.bfloat16

# model dims
S, D, H, DH, L, FF = 4096, 768, 12, 64, 4, 3072
C, W = 256, 256
P = 8                   # cores
T_OWN = S // P          # 512
T_EXT = T_OWN + 2 * C   # 1024
NJ = D // 128           # 6 feature row-tiles
NJF = FF // 128         # 24
HS = DH + 1             # 65: V head slot width (extra ones column)
HSP = 68                # fp8 V slot padded so tt-pair stride is 16B-aligned

# bias/gamma column registry (shared host/device)
PER_LAYER_COLS = 72
NB = 12 + L * PER_LAYER_COLS


def col_emb_g(j): return j
def col_emb_b(j): return 6 + j
def lbase(l): return 12 + l * PER_LAYER_COLS
def col_bq(l, j): return lbase(l) + j
def col_bk(l, j): return lbase(l) + 6 + j
def col_bo(l, j): return lbase(l) + 12 + j
def col_bff2(l, j): return lbase(l) + 18 + j
def col_bff1(l, j): return lbase(l) + 24 + j       # j in 0..23
def col_ln1g(l, j): return lbase(l) + 48 + j
def col_ln1b(l, j): return lbase(l) + 54 + j
def col_ln2g(l, j): return lbase(l) + 60 + j
def col_ln2b(l, j): return lbase(l) + 66 + j


_MAX_WAITS = 1


def _split_excess_waits(nc, max_waits=_MAX_WAITS):
    """This walrus build rejects >1 semaphore wait per instruction; move
    extras onto same-engine NoOps inserted just before."""
    n = 0
    for f in nc.m.functions:
        for bb in f.blocks:
            new_insts = []
            for inst in bb.instructions:
                si = inst.sync_info
                if si is not None and si.on_wait and len(si.on_wait) > max_waits:
                    excess = list(si.on_wait[:-max_waits])
                    keep = list(si.on_wait[-max_waits:])
                    for k, w in enumerate(excess):
                        nop = mybir.InstNoOp(name=f"{inst.name}-wsplit{k}")
                        nop.engine = inst.engine
                        nop.sync_info = mybir.SyncInfo(on_wait=[w], on_update=[])
                        new_insts.append(nop)
                        n += 1
                    inst.sync_info = mybir.SyncInfo(
                        on_wait=keep, on_update=list(si.on_update)
                    )
                new_insts.append(inst)
            bb.instructions[:] = new_insts
    return n


def _act_raw(nc, out, in_, func, bias=0.0, scale=1.0):
    """nc.scalar.activation minus the Reciprocal/Rsqrt accuracy guard."""
    eng = nc.scalar
    inputs = [eng.lower_ap(in_)]
    for arg in (bias, scale, 0.0):
        if isinstance(arg, bass.AP):
            inputs.append(eng.lower_ap(arg))
        else:
            inputs.append(mybir.ImmediateValue(dtype=mybir.dt.float32, value=arg))
    return eng.add_instruction(
        mybir.InstActivation(
            name=nc.get_next_instruction_name(),
            func=func, ins=inputs, outs=[eng.lower_ap(out)]))


def _install_ntff_hook():
    if "antenv.axon_hooks" in sys.modules:
        return
    try:
        from trn_agent_boot.trn_boot import _ntff_profile_via_ctypes
        hook = _ntff_profile_via_ctypes("/opt/axon/libaxon_pjrt.so")
    except Exception:
        hook = None
    mod = types.ModuleType("antenv.axon_hooks")
    mod.get_axon_ntff_profile_hook = lambda: hook
    mod.set_axon_ntff_profile_hook = lambda h: None
    sys.modules["antenv.axon_hooks"] = mod
    try:
        import antenv
        antenv.axon_hooks = mod
    except Exception:
        pass


# --------------------------------------------------------------------------
# device program
# --------------------------------------------------------------------------

def build_program(n_layers=L, need_mid=False):  # noqa: C901
    nc = bass.Bass("TRN2", target_bir_lowering=False, debug=False,
                   enable_asserts=True, num_devices=P)
    io = {}
    io["embT"] = nc.dram_tensor("embT", [D, T_EXT], BF16, kind="ExternalInput").ap()
    for nm in ("wq", "wk", "wv"):
        io[nm] = nc.dram_tensor(nm, [128, L, NJ, D], FP8, kind="ExternalInput").ap()
    io["wo"] = nc.dram_tensor("wo", [128, L, NJ, D], FP8, kind="ExternalInput").ap()
    io["wf1"] = nc.dram_tensor("wf1", [128, L, 4, NJ, D], FP8, kind="ExternalInput").ap()
    io["wf2"] = nc.dram_tensor("wf2", [128, L, 4, NJ, D], BF16, kind="ExternalInput").ap()
    io["bias_cols"] = nc.dram_tensor("bias_cols", [128, NB], F32, kind="ExternalInput").ap()
    io["bv_rows"] = nc.dram_tensor("bv_rows", [1, L * D], BF16, kind="ExternalInput").ap()
    nmt = 3 if need_mid else 2
    io["maskT"] = nc.dram_tensor("maskT", [128, 2, nmt, 512], BF16, kind="ExternalInput").ap()
    io["maskf"] = nc.dram_tensor("maskf", [1, T_OWN], F32, kind="ExternalInput").ap()
    io["pool_out"] = nc.dram_tensor("pool_out", [128, NJ], F32, kind="ExternalOutput").ap()

    with tile.TileContext(nc) as tc:
        _build_tile_kernel(tc, io, n_layers, need_mid)
    _split_excess_waits(nc)
    return nc


def _build_tile_kernel(tc, io, n_layers, need_mid):
    nc = tc.nc
    from contextlib import ExitStack

    ctx = ExitStack()
    with ctx:
        consts = ctx.enter_context(tc.tile_pool(name="consts", bufs=1))
        xn_pool = ctx.enter_context(tc.tile_pool(name="xn", bufs=1))
        r_pool = ctx.enter_context(tc.tile_pool(name="rp", bufs=2))
        xb_pool = ctx.enter_context(tc.tile_pool(name="xb", bufs=1))
        kqa_pool = ctx.enter_context(tc.tile_pool(name="kqa", bufs=1))
        v_pool = ctx.enter_context(tc.tile_pool(name="vp", bufs=1))
        h_pool = ctx.enter_context(tc.tile_pool(name="hp", bufs=1))
        w_pool = ctx.enter_context(tc.tile_pool(name="wp", bufs=3))
        wf1_pool = ctx.enter_context(tc.tile_pool(name="wf1p", bufs=3))
        wf2_pool = ctx.enter_context(tc.tile_pool(name="wf2p", bufs=2))
        den_pool = ctx.enter_context(tc.tile_pool(name="denp", bufs=2))
        exp_pool = ctx.enter_context(tc.tile_pool(name="expp", bufs=5))
        tmp_pool = ctx.enter_context(tc.tile_pool(name="tmpp", bufs=4))
        sq_pool = ctx.enter_context(tc.tile_pool(name="sqp", bufs=3))
        vec_pool = ctx.enter_context(tc.tile_pool(name="vecp", bufs=3))
        acc_pool = ctx.enter_context(tc.tile_pool(name="accp", bufs=1))
        dram_pool = ctx.enter_context(tc.tile_pool(name="dram", bufs=2, space="DRAM"))
        ps_pool = ctx.enter_context(tc.tile_pool(name="bigps", bufs=6, space="PSUM"))
        attn_ps = ctx.enter_context(tc.tile_pool(name="attnps", bufs=2, space="PSUM"))

        # ---- constants ----
        ones_col = consts.tile([128, 1], F32)
        nc.vector.memset(ones_col, 1.0)
        ones_row = consts.tile([1, 128], F32)
        nc.vector.memset(ones_row, 1.0)
        ones_row_bf = consts.tile([1, 128], BF16)
        nc.vector.memset(ones_row_bf, 1.0)
        ones_col_bf = consts.tile([128, 1], BF16)
        nc.vector.memset(ones_col_bf, 1.0)
        row64 = consts.tile([1, 128], F32)
        nc.vector.memset(row64, 8.0)
        bias_sb = consts.tile([128, NB], F32)
        nc.sync.dma_start(out=bias_sb, in_=io["bias_cols"])
        bv_sb = consts.tile([1, L * D], BF16)
        nc.sync.dma_start(out=bv_sb, in_=io["bv_rows"])
        mask_sb = consts.tile([128, 2, 3 if need_mid else 2, 512], BF16)
        nc.sync.dma_start(out=mask_sb, in_=io["maskT"])
        mslot = {0: 0, 1: 1, 2: 2} if need_mid else {0: 0, 2: 1}
        maskf_sb = consts.tile([1, T_OWN], F32)
        nc.sync.dma_start(out=maskf_sb, in_=io["maskf"])
        eps_col = consts.tile([1, 1], F32)
        nc.vector.memset(eps_col, 1e-5)

        def bcol(idx):
            return bias_sb[:, idx:idx + 1]

        pid = nc.partition_id()
        lidx6 = ((pid + P - 1) % P) * NJ
        ridx6 = ((pid + 1) % P) * NJ

        # ---------------- layer norm helper ----------------
        def layer_norm(src_stats, src_apply, nblk, g_col, b_col, outs,
                       ones_st=None, bw=512, blk_done=None):
            """src_*(j, blk) -> AP f32/bf16 [128,bw].
            outs(j, blk) -> list of (dst_ap, lo, hi): dst = g*t2[:, lo:hi] + b."""
            for blk in range(nblk):
                sum_ps = attn_ps.tile([1, bw], F32, tag="attn")
                sq_ps = attn_ps.tile([1, bw], F32, tag="attn")
                for j in range(NJ):
                    s = src_stats(j, blk)
                    sq = sq_pool.tile([128, 512], F32, tag="sq", bufs=2)
                    sq = sq[:, 0:bw]
                    nc.vector.tensor_tensor(sq, s, s, AluOpType.mult)
                    nc.tensor.matmul(sum_ps, ones_st or ones_col, s,
                                     start=(j == 0), stop=(j == NJ - 1))
                    nc.tensor.matmul(sq_ps, ones_col, sq,
                                     start=(j == 0), stop=(j == NJ - 1))
                mean = vec_pool.tile([1, 512], F32, tag="vec", name="mean")[:, 0:bw]
                nc.vector.tensor_scalar(mean, sum_ps, 1.0 / D, None, AluOpType.mult)
                ex2 = vec_pool.tile([1, 512], F32, tag="vec", name="ex2")[:, 0:bw]
                nc.vector.tensor_scalar(ex2, sq_ps, 1.0 / D, None, AluOpType.mult)
                var = vec_pool.tile([1, 512], F32, tag="vec", name="var")[:, 0:bw]
                nc.vector.tensor_tensor(var, mean, mean, AluOpType.mult)
                nc.vector.tensor_tensor(var, ex2, var, AluOpType.subtract)
                rstd = vec_pool.tile([1, 512], F32, tag="vec", name="rstd")[:, 0:bw]
                _act_raw(nc, rstd, var, AF.Rsqrt, bias=eps_col)
                mb = ps_pool.tile([128, 512], F32, tag="big", name="mb")[:, 0:bw]
                nc.tensor.matmul(mb, ones_row, mean, start=True, stop=True)
                rb = ps_pool.tile([128, 512], F32, tag="big", name="rb")[:, 0:bw]
                nc.tensor.matmul(rb, ones_row, rstd, start=True, stop=True)
                mb_sb = tmp_pool.tile([128, 512], F32, tag="tmp", name="mb_sb")[:, 0:bw]
                nc.vector.tensor_copy(mb_sb, mb)
                rb_sb = tmp_pool.tile([128, 512], F32, tag="tmp", name="rb_sb")[:, 0:bw]
                nc.vector.tensor_copy(rb_sb, rb)
                for j in range(NJ):
                    s = src_apply(j, blk)
                    t1 = tmp_pool.tile([128, 512], F32, tag="tmp", name="t1")[:, 0:bw]
                    nc.vector.tensor_tensor(t1, s, mb_sb, AluOpType.subtract)
                    t2 = tmp_pool.tile([128, 512], F32, tag="tmp", name="t2")[:, 0:bw]
                    nc.vector.tensor_tensor(t2, t1, rb_sb, AluOpType.mult)
                    for dst, lo, hi in outs(j, blk):
                        nc.vector.tensor_scalar(dst, t2[:, lo:hi],
                                                bcol(g_col(j)), bcol(b_col(j)),
                                                AluOpType.mult, AluOpType.add)
                if blk_done is not None:
                    blk_done(blk)

        # ---------------- embedding layer norm (over ext tokens) ----------
        xn = xn_pool.tile([128, NJ, T_EXT], FP8, tag="xn")
        r0 = r_pool.tile([128, NJ, T_OWN], BF16, tag="r")

        def emb_src(which):
            def get(j, blk):
                t = exp_pool.tile([128, 512], BF16, tag="exp", bufs=5, name="embt")
                nc.sync.dma_start(
                    out=t,
                    in_=io["embT"][j * 128:(j + 1) * 128, blk * 512:(blk + 1) * 512])
                return t
            return get

        def emb_outs(j, blk):
            dsts = [(xn[:, j, blk * 512:(blk + 1) * 512], 0, 512)]
            if blk == 0:
                dsts.append((r0[:, j, 0:256], 256, 512))
            else:
                dsts.append((r0[:, j, 256:512], 0, 256))
            return dsts

        with nc.named_scope("emb_ln"):
            layer_norm(emb_src(0), emb_src(1), 2, col_emb_g, col_emb_b, emb_outs,
                       ones_st=ones_col_bf)

        # ---------------- transformer layers ----------------
        def load_qkvo(l):
            ws = []
            for nm in ("wq", "wk", "wv", "wo"):
                w = w_pool.tile([128, NJ, D], FP8, tag="w8", bufs=8,
                                name=f"{nm}{l}")
                nc.sync.dma_start(out=w, in_=io[nm][:, l])
                ws.append(w)
            return ws

        wtiles = load_qkvo(0)
        for l in range(n_layers):
            first = (l == 0)
            last = (l == n_layers - 1)

            wq_sb, wk_sb, wv_sb, wo_sb = wtiles

            kT = kqa_pool.tile([128, NJ, T_EXT], BF16, tag="kT")
            qT = kqa_pool.tile([128, NJ, T_OWN], BF16, tag="qT")
            v_sb = v_pool.tile([128, 8, H, HSP], FP8, tag="v")
            nc.vector.memset(v_sb[:, :, :, DH:HS], 1.0)

            def k_proj(tlo, thi):
                nn = thi - tlo
                for mj in range(NJ):
                    ps = ps_pool.tile([128, 512], F32, tag="big")
                    for kj in range(0, NJ, 2):
                        nc.tensor.matmul(
                            ps[:, 0:nn], wk_sb[:, kj:kj + 2, mj * 128:(mj + 1) * 128],
                            xn[:, kj:kj + 2, tlo:thi],
                            start=(kj == 0), stop=(kj == NJ - 2),
                            perf_mode=mybir.MatmulPerfMode.DoubleRow)
                    nc.vector.tensor_scalar(kT[:, mj, tlo:thi], ps[:, 0:nn],
                                            1.0 / FP8_WSCALE, bcol(col_bk(l, mj)),
                                            AluOpType.mult, AluOpType.add)

            def v_proj(tts):
                for tt in tts:
                    for ob in range(2):
                        psfull = ps_pool.tile([128, 512], F32, tag="big")
                        ps = psfull[:, 0:384]
                        for kj in range(0, NJ, 2):
                            nc.tensor.matmul(
                                ps, xn[:, kj:kj + 2, tt * 128:(tt + 1) * 128],
                                wv_sb[:, kj:kj + 2, ob * 384:(ob + 1) * 384],
                                start=(kj == 0), stop=False,
                                perf_mode=mybir.MatmulPerfMode.DoubleRow)
                        nc.tensor.matmul(
                            ps, ones_row_bf,
                            bv_sb[:, l * D + ob * 384: l * D + (ob + 1) * 384],
                            start=False, stop=True)
                        nc.vector.tensor_scalar(
                            v_sb[:, tt, ob * 6:(ob + 1) * 6, 0:DH],
                            ps.rearrange("p (h s) -> p h s", s=DH),
                            1.0 / (FP8_WSCALE * 8.0), None, AluOpType.mult)

            # -- own-token projections (independent of the halo AllGather) --
            with nc.named_scope(f"L{l}.qkv_own"):
                for mj in range(NJ):
                    ps = ps_pool.tile([128, 512], F32, tag="big")
                    for kj in range(0, NJ, 2):
                        nc.tensor.matmul(
                            ps, wq_sb[:, kj:kj + 2, mj * 128:(mj + 1) * 128],
                            xn[:, kj:kj + 2, 256:768],
                            start=(kj == 0), stop=(kj == NJ - 2),
                            perf_mode=mybir.MatmulPerfMode.DoubleRow)
                    nc.vector.tensor_scalar(qT[:, mj, :], ps,
                                            1.0 / FP8_WSCALE, bcol(col_bq(l, mj)),
                                            AluOpType.mult, AluOpType.add)
                if first:
                    k_proj(0, 512)
                    k_proj(512, 1024)
                    v_proj(range(8))
                else:
                    k_proj(256, 768)
                    v_proj(range(2, 6))

            # -- attention (heads software-pipelined) --
            attnT = kqa_pool.tile([128, NJ, T_OWN], FP8, tag="attnT")

            def emit_scores(n, h):
                jh, po = h // 2, (h % 2) * 64
                ems = []
                for t in range(3):
                    ps = ps_pool.tile([128, 512], F32, tag="big")
                    for half in range(2):
                        kofs = n * 256 + (2 * t + half) * 128
                        nc.tensor.matmul(
                            ps[:, half * 256:(half + 1) * 256],
                            kT[po:po + 64, jh, kofs:kofs + 128],
                            qT[po:po + 64, jh, n * 256:(n + 1) * 256],
                            start=True, stop=True)
                    e = exp_pool.tile([128, 512], FP8, tag="exp")
                    nc.scalar.activation(e, ps, AF.Exp)
                    if t == 1 and not need_mid:
                        ems.append(e)
                    else:
                        em = exp_pool.tile([128, 512], FP8, tag="em")
                        nc.vector.tensor_tensor(
                            em, e, mask_sb[:, n, mslot[t], :], AluOpType.mult)
                        ems.append(em)
                return ems

            dens = [den_pool.tile([1, H * 256], BF16, tag="den", name=f"den{i}")
                    for i in range(2)]

            def emit_av(n, h, ems):
                jh, po = h // 2, (h % 2) * 64
                aps = attn_ps.tile([HS, 256], F32, tag="attn")
                for t in range(3):
                    tt = n * 2 + 2 * t
                    nc.tensor.matmul(
                        aps, v_sb[:, tt:tt + 2, h, 0:HS],
                        ems[t].rearrange("p (k q) -> p k q", k=2),
                        start=(t == 0), stop=(t == 2),
                        perf_mode=mybir.MatmulPerfMode.DoubleRow)
                nc.vector.tensor_copy(dens[n][:, h * 256:(h + 1) * 256],
                                      aps[64:65, :])
                nc.vector.tensor_copy(
                    attnT[po:po + 64, jh, n * 256:(n + 1) * 256], aps[0:64, :])

            def emit_norm(n):
                # grouped reciprocals: one ACT table load for the whole block
                for h in range(H):
                    jh, po = h // 2, (h % 2) * 64
                    rec = vec_pool.tile([1, 256], F32, tag="rec")
                    _act_raw(nc, rec, dens[n][:, h * 256:(h + 1) * 256],
                             AF.Reciprocal)
                    bc = attn_ps.tile([64, 256], F32, tag="attn")
                    nc.tensor.matmul(bc, row64[0:1, 0:64], rec,
                                     start=True, stop=True)
                    sl = attnT[po:po + 64, jh, n * 256:(n + 1) * 256]
                    nc.vector.tensor_tensor(sl, sl, bc, AluOpType.mult)

            pend = None
            for n in range(2):
                if n == 1:
                    if not first:
                        with nc.named_scope(f"L{l}.halo_right"):
                            k_proj(768, 1024)
                            v_proj(range(6, 8))
                    if not last:
                        wtiles = load_qkvo(l + 1)
                with nc.named_scope(f"L{l}.attn{n}"):
                    if n == 0 and not first:
                        with nc.named_scope(f"L{l}.halo_left"):
                            k_proj(0, 256)
                            v_proj(range(0, 2))
                    for h in range(H):
                        ems = emit_scores(n, h)
                        if pend is not None:
                            emit_av(*pend)
                        pend = (n, h, ems)
                        if n == 1 and h == 1:
                            emit_norm(0)
            emit_av(*pend)
            emit_norm(1)

            # -- Wo projection + residual -> r1 --
            r1 = r_pool.tile([128, NJ, T_OWN], BF16, tag="r")
            with nc.named_scope(f"L{l}.wo"):
                for mj in range(NJ):
                    ps = ps_pool.tile([128, 512], F32, tag="big")
                    for kj in range(0, NJ, 2):
                        nc.tensor.matmul(
                            ps, wo_sb[:, kj:kj + 2, mj * 128:(mj + 1) * 128],
                            attnT[:, kj:kj + 2, :],
                            start=(kj == 0), stop=(kj == NJ - 2),
                            perf_mode=mybir.MatmulPerfMode.DoubleRow)
                    t = sq_pool.tile([128, 512], F32, tag="ao", bufs=2)
                    nc.scalar.activation(t, ps, AF.Identity,
                                         bias=bcol(col_bo(l, mj)),
                                         scale=1.0 / FP8_WSCALE)
                    nc.vector.tensor_tensor(r1[:, mj, :], t, r0[:, mj, :],
                                            AluOpType.add)

            # -- LN1 -> xn1b (bf16; also the FFN residual) --
            xn1b = xb_pool.tile([128, NJ, T_OWN], BF16, tag="xn1b")
            xn1b8 = xb_pool.tile([128, NJ, T_OWN], FP8, tag="xn1b8")
            with nc.named_scope(f"L{l}.ln1"):
                layer_norm(
                    lambda j, blk: r1[:, j, :], lambda j, blk: r1[:, j, :], 1,
                    lambda j: col_ln1g(l, j), lambda j: col_ln1b(l, j),
                    lambda j, blk: [(xn1b[:, j, :], 0, 512),
                                    (xn1b8[:, j, :], 0, 512)],
                    ones_st=ones_col_bf)

            # -- FFN --
            r2 = r_pool.tile([128, NJ, T_OWN], BF16, tag="r")
            with nc.named_scope(f"L{l}.ffn"):
                f2ps = [ps_pool.tile([128, 512], F32, tag="big",
                                     name=f"f2ps{mj}") for mj in range(NJ)]
                for q in range(4):
                    wf1_sb = wf1_pool.tile([128, NJ, D], FP8, tag="wf1")
                    nc.gpsimd.dma_start(out=wf1_sb, in_=io["wf1"][:, l, q])
                    wf2_sb = wf2_pool.tile([128, NJ, D], BF16, tag="wf2")
                    nc.gpsimd.dma_start(out=wf2_sb, in_=io["wf2"][:, l, q])
                    hq = h_pool.tile([128, NJ, T_OWN], BF16, tag="h", bufs=2)
                    for mj6 in range(NJ):
                        ps = attn_ps.tile([128, 512], F32, tag="attn")
                        for kj in range(0, NJ, 2):
                            nc.tensor.matmul(
                                ps, wf1_sb[:, kj:kj + 2, mj6 * 128:(mj6 + 1) * 128],
                                xn1b8[:, kj:kj + 2, :],
                                start=(kj == 0), stop=(kj == NJ - 2),
                                perf_mode=mybir.MatmulPerfMode.DoubleRow)
                        nc.scalar.activation(
                            hq[:, mj6, :], ps, AF.Gelu,
                            bias=bcol(col_bff1(l, q * NJ + mj6)),
                            scale=1.0 / FP8_WSCALE)
                    for mj in range(NJ):
                        for kj in range(NJ):
                            nc.tensor.matmul(
                                f2ps[mj], wf2_sb[:, kj, mj * 128:(mj + 1) * 128],
                                hq[:, kj, :],
                                start=(q == 0 and kj == 0),
                                stop=(q == 3 and kj == NJ - 1))
                for mj in range(NJ):
                    t = sq_pool.tile([128, 512], F32, tag="ao", bufs=2)
                    nc.scalar.activation(t, f2ps[mj], AF.Identity,
                                         bias=bcol(col_bff2(l, mj)))
                    nc.vector.tensor_tensor(r2[:, mj, :], t, xn1b[:, mj, :],
                                            AluOpType.add)

            # -- LN2 -> next xn (+ next r0), in halves; right half first so
            # its AllGather (which feeds the left halos) launches early --
            xn_next = None if last else xn_pool.tile([128, NJ, T_EXT], FP8, tag="xn")
            xn2f = r_pool.tile([128, NJ, T_OWN], BF16, tag="r")
            HLO = (256, 0)  # blk0 = own tokens 256:512, blk1 = 0:256

            def ln2_outs(j, blk, xn_next=xn_next, xn2f=xn2f, last=last):
                lo = HLO[blk]
                dsts = [(xn2f[:, j, lo:lo + 256], 0, 256)]
                if not last:
                    dsts.append((xn_next[:, j, 256 + lo:512 + lo], 0, 256))
                return dsts

            def ln2_blk_done(blk, xn_next=xn_next, last=last, l=l):
                if last:
                    return
                with nc.named_scope(f"L{l}.allgather{blk}"):
                    agi = dram_pool.tile([D, 256], FP8, tag="agi")
                    ago = dram_pool.tile([P * D, 256], FP8, tag="ago",
                                         addr_space="Shared")
                    lo = HLO[blk]
                    nc.sync.dma_start(
                        out=agi.rearrange("(j p) t -> p j t", p=128),
                        in_=xn_next[:, :, 256 + lo:512 + lo])
                    nc.gpsimd.collective_compute(
                        "AllGather", AluOpType.bypass,
                        replica_groups=[list(range(P))],
                        ins=[agi.opt()], outs=[ago.opt()])
                    agv = ago.rearrange("(r j p) t -> p (r j) t", j=NJ, p=128)
                    if blk == 0:
                        nc.sync.dma_start(out=xn_next[:, :, 0:256],
                                          in_=agv[:, bass.ds(lidx6, NJ), :])
                    else:
                        nc.sync.dma_start(out=xn_next[:, :, 768:1024],
                                          in_=agv[:, bass.ds(ridx6, NJ), :])

            with nc.named_scope(f"L{l}.ln2"):
                layer_norm(
                    lambda j, blk, r2=r2: r2[:, j, HLO[blk]:HLO[blk] + 256],
                    lambda j, blk, r2=r2: r2[:, j, HLO[blk]:HLO[blk] + 256], 2,
                    lambda j: col_ln2g(l, j), lambda j: col_ln2b(l, j),
                    ln2_outs, ones_st=ones_col_bf, bw=256,
                    blk_done=ln2_blk_done)

            if not last:
                xn = xn_next
            r0 = xn2f

        # ---------------- pooling partials ----------------
        with nc.named_scope("pool"):
            accs = acc_pool.tile([128, NJ], F32, tag="accs")
            if need_mid:
                mb = ps_pool.tile([128, 512], F32, tag="big")
                nc.tensor.matmul(mb, ones_row, maskf_sb, start=True, stop=True)
                for j in range(NJ):
                    mskd = tmp_pool.tile([128, 512], F32, tag="tmp")
                    nc.vector.tensor_tensor(mskd, r0[:, j, :], mb, AluOpType.mult)
                    scr = sq_pool.tile([128, 512], F32, tag="sq", bufs=2)
                    nc.scalar.activation(scr, mskd, AF.Copy,
                                         accum_out=accs[:, j:j + 1])
            else:
                for j in range(NJ):
                    scr = sq_pool.tile([128, 512], F32, tag="sq", bufs=2)
                    nc.scalar.activation(scr, r0[:, j, :], AF.Copy,
                                         accum_out=accs[:, j:j + 1])
            nc.sync.dma_start(out=io["pool_out"], in_=accs)


# --------------------------------------------------------------------------
# host side
# --------------------------------------------------------------------------

def _build_masks(attention_mask):
    """[P, 2, 3*C, C] multiplicative float mask (band + attn mask + edges)."""
    maskf = np.asarray(attention_mask, np.float32).reshape(S)
    masks = np.zeros((P, 2, 3 * C, C), np.float32)
    qi = np.arange(C)[None, :]
    kj = np.arange(3 * C)[:, None]
    band = (np.abs(kj - C - qi) <= W)
    for c in range(P):
        for n in range(2):
            g0 = c * T_OWN + n * C
            kg = g0 - C + np.arange(3 * C)
            valid = (kg >= 0) & (kg < S)
            mvals = np.where(valid, maskf[np.clip(kg, 0, S - 1)], 0.0)
            masks[c, n] = band * (mvals[:, None] > 0)
    return masks


_cache = {}


def kernel(input_ids, attention_mask, word_emb, pos_emb, emb_g, emb_b,
           Wq, Wk, Wv, Wo, bq, bk, bv, bo, ln1_g, ln1_b,
           Wff1, bff1, Wff2, bff2, ln2_g, ln2_b,
           W1, b1, W2, b2, W3, b3):
    to32 = lambda a: np.ascontiguousarray(np.asarray(a, np.float32))
    tob = lambda a: np.asarray(a, np.float32).astype(NPBF16)
    to8 = lambda a: np.clip(np.asarray(a, np.float32) * FP8_WSCALE,
                            -240, 240).astype(ml_dtypes.float8_e4m3)
    ids = np.asarray(input_ids).reshape(S)
    word_emb, pos_emb = to32(word_emb), to32(pos_emb)
    emb = word_emb[ids] + pos_emb                      # [S, D] host gather
    masks = _build_masks(attention_mask)
    maskf = np.asarray(attention_mask, np.float32).reshape(S)
    need_mid = not bool(np.asarray(attention_mask).all())

    scale = 1.0 / np.sqrt(np.float32(DH))
    wq_s = to32(Wq) * scale
    bq_s = to32(bq) * scale

    bias_cols = np.zeros((128, NB), np.float32)
    for j in range(NJ):
        sl = slice(j * 128, (j + 1) * 128)
        bias_cols[:, col_emb_g(j)] = to32(emb_g)[sl]
        bias_cols[:, col_emb_b(j)] = to32(emb_b)[sl]
    for l in range(L):
        for j in range(NJ):
            sl = slice(j * 128, (j + 1) * 128)
            bias_cols[:, col_bq(l, j)] = bq_s[l][sl]
            bias_cols[:, col_bk(l, j)] = to32(bk)[l][sl]
            bias_cols[:, col_bo(l, j)] = to32(bo)[l][sl]
            bias_cols[:, col_bff2(l, j)] = to32(bff2)[l][sl]
            bias_cols[:, col_ln1g(l, j)] = to32(ln1_g)[l][sl]
            bias_cols[:, col_ln1b(l, j)] = to32(ln1_b)[l][sl]
            bias_cols[:, col_ln2g(l, j)] = to32(ln2_g)[l][sl]
            bias_cols[:, col_ln2b(l, j)] = to32(ln2_b)[l][sl]
        for j in range(NJF):
            bias_cols[:, col_bff1(l, j)] = to32(bff1)[l][j * 128:(j + 1) * 128]

    # weights pre-transposed host-side to [128, L, (q,) kj, out] so each
    # per-layer DMA reads one contiguous run per partition
    wq_b = np.ascontiguousarray(
        to8(wq_s).reshape(L, NJ, 128, D).transpose(2, 0, 1, 3))
    wk_b = np.ascontiguousarray(
        to8(Wk).reshape(L, NJ, 128, D).transpose(2, 0, 1, 3))
    wv_b = np.ascontiguousarray(
        to8(Wv).reshape(L, NJ, 128, D).transpose(2, 0, 1, 3))
    wo_b = np.ascontiguousarray(
        to8(Wo).reshape(L, NJ, 128, D).transpose(2, 0, 1, 3))
    wf1_b = np.ascontiguousarray(
        to8(Wff1).reshape(L, NJ, 128, 4, D).transpose(2, 0, 3, 1, 4))
    wf2_b = np.ascontiguousarray(
        tob(Wff2).reshape(L, 4, NJ, 128, D).transpose(3, 0, 1, 2, 4))
    bv_b = np.ascontiguousarray(tob(np.asarray(bv, np.float32)
                                    * FP8_WSCALE).reshape(1, L * D))

    n_layers = int(os.environ.get("KERNEL_LAYERS", L))
    key = (n_layers, need_mid)
    if key not in _cache:
        _cache[key] = build_program(n_layers, need_mid)
    nc = _cache[key]

    in_maps = []
    for c in range(P):
        lo, hi = c * T_OWN - C, c * T_OWN + T_OWN + C
        e = np.zeros((T_EXT, D), np.float32)
        s0, s1 = max(lo, 0), min(hi, S)
        e[s0 - lo:s1 - lo] = emb[s0:s1]
        mp = np.zeros((2, 3, 128, 512), np.float32)
        for n in range(2):
            for t in range(3):
                for half in range(2):
                    mp[n, t, :, half * 256:(half + 1) * 256] = \
                        masks[c, n][(2 * t + half) * 128:(2 * t + half + 1) * 128, :]
        in_maps.append({
            "embT": np.ascontiguousarray(e.T.astype(NPBF16)),
            "wq": wq_b, "wk": wk_b, "wv": wv_b, "wo": wo_b,
            "wf1": wf1_b, "wf2": wf2_b,
            "bias_cols": bias_cols,
            "bv_rows": bv_b,
            "maskT": np.ascontiguousarray(
                mp.transpose(2, 0, 1, 3)[:, :, ([0, 1, 2] if need_mid else [0, 2])]
                .astype(NPBF16)),
            "maskf": np.ascontiguousarray(
                maskf[c * T_OWN:(c + 1) * T_OWN].reshape(1, T_OWN)),
        })

    trace = os.environ.get("KERNEL_TRACE", "0") == "1"
    if trace:
        _install_ntff_hook()
    res = run_bass_kernel_spmd(nc, in_maps, core_ids=list(range(P)), trace=trace)
    kernel.last_exec_time_ns = res.exec_time_ns
    kernel.last_results = res.results
    kernel.last_res = res

    pooled = np.zeros(D, np.float64)
    for c in range(P):
        po = np.asarray(res.results[c]["pool_out"], np.float64)   # [128, NJ]
        pooled += po.T.reshape(D)                                 # f = j*128+p
    msum = max(maskf.sum(), 1e-9)
    pooled = (pooled / msum).astype(np.float32)

    h1 = np.maximum(pooled @ to32(W1) + to32(b1), 0)
    h2 = np.maximum(h1 @ to32(W2) + to32(b2), 0)
    pred = (h2 @ to32(W3) + to32(b3))[None].astype(np.float32)
    return pred, pred


kernel.last_exec_time_ns = None
kernel.last_results = None
kernel.last_res = None


# revision 30
# speedup vs baseline: 1.0645x; 1.0005x over previous
"""Trainium2 Bass kernel for the sliding-window-attention transformer
(nn_Model_22728966930624).

Sharding: sequence-parallel over 8 NeuronCores. Core c owns tokens
[c*512, (c+1)*512); each layer's K/V are computed over an extended region
with a 256-token halo on each side. Halos are refreshed between layers with
two staggered 8-rank AllGathers (fp8, right half first so the left halos —
needed first by attention block 0 — land early); both overlap the next
layer's own-token Q/K/V projections.

Precision: QKV/Wo/FFN1 weights and their activations run in fp8-e4m3 with
DoubleRow matmuls (weights prescaled x64 host-side; V additionally scaled
1/64 so the unnormalized fp8 attention accumulator stays inside TRN e4m3's
+-240 range — the reciprocal broadcast multiplies by 64/denominator to
compensate). FFN2 and the residual stream stay bf16: e4m3 noise through
GELU/LayerNorm nonlinearities rectifies into a systematic bias that costs
~1.4e-2 rel err if FFN2 is quantized. Softmax reciprocals and LayerNorm
rsqrt run on the scalar engine via raw-emitted Reciprocal/Rsqrt activations
(measured ~1e-5 accurate), grouped per attention block so the activation
table is not thrashed.

Device layout: activations are feature-major ("transposed") in SBUF:
x^T [768 rows -> 6 tiles x 128 partitions, tokens in the free dim].
LayerNorm statistics are computed with ones-matmuls on the PE; per-token
mean/rstd are broadcast across partitions with K=1 matmuls into PSUM.
Softmax is computed max-free (scores are O(1) by construction) with a
multiplicative band mask, and the softmax denominator comes for free from
an extra ones-column appended to each attention head's V block. The
attention head loop is software-pipelined (scores of head h+1 are emitted
before the PV matmuls of head h) so the PE never waits on exp/mask.
"""
import os
import sys
import types

import numpy as np
import ml_dtypes

import concourse.bass as bass
import concourse.mybir as mybir
import concourse.tile as tile
from concourse.alu_op_type import AluOpType
from concourse.bass_utils import run_bass_kernel_spmd

F32 = mybir.dt.float32
BF16 = mybir.dt.bfloat16
FP8 = mybir.dt.float8e4
FP8_WSCALE = 64.0
AF = mybir.ActivationFunctionType
NPBF16 = ml_dtypes.bfloat16

# model dims
S, D, H, DH, L, FF = 4096, 768, 12, 64, 4, 3072
C, W = 256, 256
P = 8                   # cores
T_OWN = S // P          # 512
T_EXT = T_OWN + 2 * C   # 1024
NJ = D // 128           # 6 feature row-tiles
NJF = FF // 128         # 24
HS = DH + 1             # 65: V head slot width (extra ones column)
HSP = 68                # fp8 V slot padded so tt-pair stride is 16B-aligned

# bias/gamma column registry (shared host/device)
PER_LAYER_COLS = 72
NB = 12 + L * PER_LAYER_COLS


def col_emb_g(j): return j
def col_emb_b(j): return 6 + j
def lbase(l): return 12 + l * PER_LAYER_COLS
def col_bq(l, j): return lbase(l) + j
def col_bk(l, j): return lbase(l) + 6 + j
def col_bo(l, j): return lbase(l) + 12 + j
def col_bff2(l, j): return lbase(l) + 18 + j
def col_bff1(l, j): return lbase(l) + 24 + j       # j in 0..23
def col_ln1g(l, j): return lbase(l) + 48 + j
def col_ln1b(l, j): return lbase(l) + 54 + j
def col_ln2g(l, j): return lbase(l) + 60 + j
def col_ln2b(l, j): return lbase(l) + 66 + j


_MAX_WAITS = 1


def _split_excess_waits(nc, max_waits=_MAX_WAITS):
    """This walrus build rejects >1 semaphore wait per instruction; move
    extras onto same-engine NoOps inserted just before."""
    n = 0
    for f in nc.m.functions:
        for bb in f.blocks:
            new_insts = []
            for inst in bb.instructions:
                si = inst.sync_info
                if si is not None and si.on_wait and len(si.on_wait) > max_waits:
                    excess = list(si.on_wait[:-max_waits])
                    keep = list(si.on_wait[-max_waits:])
                    for k, w in enumerate(excess):
                        nop = mybir.InstNoOp(name=f"{inst.name}-wsplit{k}")
                        nop.engine = inst.engine
                        nop.sync_info = mybir.SyncInfo(on_wait=[w], on_update=[])
                        new_insts.append(nop)
                        n += 1
                    inst.sync_info = mybir.SyncInfo(
                        on_wait=keep, on_update=list(si.on_update)
                    )
                new_insts.append(inst)
            bb.instructions[:] = new_insts
    return n


def _act_raw(nc, out, in_, func, bias=0.0, scale=1.0):
    """nc.scalar.activation minus the Reciprocal/Rsqrt accuracy guard."""
    eng = nc.scalar
    inputs = [eng.lower_ap(in_)]
    for arg in (bias, scale, 0.0):
        if isinstance(arg, bass.AP):
            inputs.append(eng.lower_ap(arg))
        else:
            inputs.append(mybir.ImmediateValue(dtype=mybir.dt.float32, value=arg))
    return eng.add_instruction(
        mybir.InstActivation(
            name=nc.get_next_instruction_name(),
            func=func, ins=inputs, outs=[eng.lower_ap(out)]))


def _install_ntff_hook():
    if "antenv.axon_hooks" in sys.modules:
        return
    try:
        from trn_agent_boot.trn_boot import _ntff_profile_via_ctypes
        hook = _ntff_profile_via_ctypes("/opt/axon/libaxon_pjrt.so")
    except Exception:
        hook = None
    mod = types.ModuleType("antenv.axon_hooks")
    mod.get_axon_ntff_profile_hook = lambda: hook
    mod.set_axon_ntff_profile_hook = lambda h: None
    sys.modules["antenv.axon_hooks"] = mod
    try:
        import antenv
        antenv.axon_hooks = mod
    except Exception:
        pass


# --------------------------------------------------------------------------
# device program
# --------------------------------------------------------------------------

def build_program(n_layers=L, need_mid=False):  # noqa: C901
    nc = bass.Bass("TRN2", target_bir_lowering=False, debug=False,
                   enable_asserts=True, num_devices=P)
    io = {}
    io["embT"] = nc.dram_tensor("embT", [D, T_EXT], BF16, kind="ExternalInput").ap()
    for nm in ("wq", "wk", "wv"):
        io[nm] = nc.dram_tensor(nm, [128, L, NJ, D], FP8, kind="ExternalInput").ap()
    io["wo"] = nc.dram_tensor("wo", [128, L, NJ, D], FP8, kind="ExternalInput").ap()
    io["wf1"] = nc.dram_tensor("wf1", [128, L, 4, NJ, D], FP8, kind="ExternalInput").ap()
    io["wf2"] = nc.dram_tensor("wf2", [128, L, 4, NJ, D], BF16, kind="ExternalInput").ap()
    io["bias_cols"] = nc.dram_tensor("bias_cols", [128, NB], F32, kind="ExternalInput").ap()
    io["bv_rows"] = nc.dram_tensor("bv_rows", [1, L * D], BF16, kind="ExternalInput").ap()
    nmt = 3 if need_mid else 2
    io["maskT"] = nc.dram_tensor("maskT", [128, 2, nmt, 512], BF16, kind="ExternalInput").ap()
    io["maskf"] = nc.dram_tensor("maskf", [1, T_OWN], F32, kind="ExternalInput").ap()
    io["pool_out"] = nc.dram_tensor("pool_out", [128, NJ], F32, kind="ExternalOutput").ap()

    with tile.TileContext(nc) as tc:
        _build_tile_kernel(tc, io, n_layers, need_mid)
    _split_excess_waits(nc)
    return nc


def _build_tile_kernel(tc, io, n_layers, need_mid):
    nc = tc.nc
    from contextlib import ExitStack

    ctx = ExitStack()
    with ctx:
        consts = ctx.enter_context(tc.tile_pool(name="consts", bufs=1))
        xn_pool = ctx.enter_context(tc.tile_pool(name="xn", bufs=1))
        r_pool = ctx.enter_context(tc.tile_pool(name="rp", bufs=2))
        xb_pool = ctx.enter_context(tc.tile_pool(name="xb", bufs=1))
        kqa_pool = ctx.enter_context(tc.tile_pool(name="kqa", bufs=1))
        v_pool = ctx.enter_context(tc.tile_pool(name="vp", bufs=1))
        h_pool = ctx.enter_context(tc.tile_pool(name="hp", bufs=1))
        w_pool = ctx.enter_context(tc.tile_pool(name="wp", bufs=3))
        wf1_pool = ctx.enter_context(tc.tile_pool(name="wf1p", bufs=3))
        wf2_pool = ctx.enter_context(tc.tile_pool(name="wf2p", bufs=2))
        den_pool = ctx.enter_context(tc.tile_pool(name="denp", bufs=2))
        exp_pool = ctx.enter_context(tc.tile_pool(name="expp", bufs=5))
        tmp_pool = ctx.enter_context(tc.tile_pool(name="tmpp", bufs=4))
        sq_pool = ctx.enter_context(tc.tile_pool(name="sqp", bufs=3))
        vec_pool = ctx.enter_context(tc.tile_pool(name="vecp", bufs=3))
        acc_pool = ctx.enter_context(tc.tile_pool(name="accp", bufs=1))
        dram_pool = ctx.enter_context(tc.tile_pool(name="dram", bufs=2, space="DRAM"))
        ps_pool = ctx.enter_context(tc.tile_pool(name="bigps", bufs=6, space="PSUM"))
        attn_ps = ctx.enter_context(tc.tile_pool(name="attnps", bufs=2, space="PSUM"))

        # ---- constants ----
        ones_col = consts.tile([128, 1], F32)
        nc.vector.memset(ones_col, 1.0)
        ones_row = consts.tile([1, 128], F32)
        nc.vector.memset(ones_row, 1.0)
        ones_row_bf = consts.tile([1, 128], BF16)
        nc.vector.memset(ones_row_bf, 1.0)
        ones_col_bf = consts.tile([128, 1], BF16)
        nc.vector.memset(ones_col_bf, 1.0)
        row64 = consts.tile([1, 128], F32)
        nc.vector.memset(row64, 8.0)
        bias_sb = consts.tile([128, NB], F32)
        nc.sync.dma_start(out=bias_sb, in_=io["bias_cols"])
        bv_sb = consts.tile([1, L * D], BF16)
        nc.sync.dma_start(out=bv_sb, in_=io["bv_rows"])
        mask_sb = consts.tile([128, 2, 3 if need_mid else 2, 512], BF16)
        nc.sync.dma_start(out=mask_sb, in_=io["maskT"])
        mslot = {0: 0, 1: 1, 2: 2} if need_mid else {0: 0, 2: 1}
        maskf_sb = consts.tile([1, T_OWN], F32)
        nc.sync.dma_start(out=maskf_sb, in_=io["maskf"])
        eps_col = consts.tile([1, 1], F32)
        nc.vector.memset(eps_col, 1e-5)

        def bcol(idx):
            return bias_sb[:, idx:idx + 1]

        pid = nc.partition_id()
        lidx6 = ((pid + P - 1) % P) * NJ
        ridx6 = ((pid + 1) % P) * NJ

        # ---------------- layer norm helper ----------------
        def layer_norm(src_stats, src_apply, nblk, g_col, b_col, outs,
                       ones_st=None, bw=512, blk_done=None):
            """src_*(j, blk) -> AP f32/bf16 [128,bw].
            outs(j, blk) -> list of (dst_ap, lo, hi): dst = g*t2[:, lo:hi] + b."""
            for blk in range(nblk):
                sum_ps = attn_ps.tile([1, bw], F32, tag="attn")
                sq_ps = attn_ps.tile([1, bw], F32, tag="attn")
                for j in range(NJ):
                    s = src_stats(j, blk)
                    sq = sq_pool.tile([128, 512], F32, tag="sq", bufs=2)
                    sq = sq[:, 0:bw]
                    nc.vector.tensor_tensor(sq, s, s, AluOpType.mult)
                    nc.tensor.matmul(sum_ps, ones_st or ones_col, s,
                                     start=(j == 0), stop=(j == NJ - 1))
                    nc.tensor.matmul(sq_ps, ones_col, sq,
                                     start=(j == 0), stop=(j == NJ - 1))
                mean = vec_pool.tile([1, 512], F32, tag="vec", name="mean")[:, 0:bw]
                nc.vector.tensor_scalar(mean, sum_ps, 1.0 / D, None, AluOpType.mult)
                ex2 = vec_pool.tile([1, 512], F32, tag="vec", name="ex2")[:, 0:bw]
                nc.vector.tensor_scalar(ex2, sq_ps, 1.0 / D, None, AluOpType.mult)
                var = vec_pool.tile([1, 512], F32, tag="vec", name="var")[:, 0:bw]
                nc.vector.tensor_tensor(var, mean, mean, AluOpType.mult)
                nc.vector.tensor_tensor(var, ex2, var, AluOpType.subtract)
                rstd = vec_pool.tile([1, 512], F32, tag="vec", name="rstd")[:, 0:bw]
                _act_raw(nc, rstd, var, AF.Rsqrt, bias=eps_col)
                mb = ps_pool.tile([128, 512], F32, tag="big", name="mb")[:, 0:bw]
                nc.tensor.matmul(mb, ones_row, mean, start=True, stop=True)
                rb = ps_pool.tile([128, 512], F32, tag="big", name="rb")[:, 0:bw]
                nc.tensor.matmul(rb, ones_row, rstd, start=True, stop=True)
                mb_sb = tmp_pool.tile([128, 512], F32, tag="tmp", name="mb_sb")[:, 0:bw]
                nc.vector.tensor_copy(mb_sb, mb)
                rb_sb = tmp_pool.tile([128, 512], F32, tag="tmp", name="rb_sb")[:, 0:bw]
                nc.vector.tensor_copy(rb_sb, rb)
                for j in range(NJ):
                    s = src_apply(j, blk)
                    t1 = tmp_pool.tile([128, 512], F32, tag="tmp", name="t1")[:, 0:bw]
                    nc.vector.tensor_tensor(t1, s, mb_sb, AluOpType.subtract)
                    t2 = tmp_pool.tile([128, 512], F32, tag="tmp", name="t2")[:, 0:bw]
                    nc.vector.tensor_tensor(t2, t1, rb_sb, AluOpType.mult)
                    for dst, lo, hi in outs(j, blk):
                        nc.vector.tensor_scalar(dst, t2[:, lo:hi],
                                                bcol(g_col(j)), bcol(b_col(j)),
                                                AluOpType.mult, AluOpType.add)
                if blk_done is not None:
                    blk_done(blk)

        # ---------------- embedding layer norm (over ext tokens) ----------
        xn = xn_pool.tile([128, NJ, T_EXT], FP8, tag="xn")
        r0 = r_pool.tile([128, NJ, T_OWN], BF16, tag="r")

        emb_tiles = []
        for j in range(NJ):
            t = exp_pool.tile([128, T_EXT], BF16, tag="embt", bufs=6,
                              name=f"embt{j}")
            nc.sync.dma_start(out=t, in_=io["embT"][j * 128:(j + 1) * 128, :])
            emb_tiles.append(t)

        def emb_src(which):
            def get(j, blk):
                return emb_tiles[j][:, blk * 512:(blk + 1) * 512]
            return get

        def emb_outs(j, blk):
            dsts = [(xn[:, j, blk * 512:(blk + 1) * 512], 0, 512)]
            if blk == 0:
                dsts.append((r0[:, j, 0:256], 256, 512))
            else:
                dsts.append((r0[:, j, 256:512], 0, 256))
            return dsts

        with nc.named_scope("emb_ln"):
            layer_norm(emb_src(0), emb_src(1), 2, col_emb_g, col_emb_b, emb_outs,
                       ones_st=ones_col_bf)

        # ---------------- transformer layers ----------------
        def load_qkvo(l):
            ws = []
            for nm in ("wq", "wk", "wv", "wo"):
                w = w_pool.tile([128, NJ, D], FP8, tag="w8", bufs=8,
                                name=f"{nm}{l}")
                nc.scalar.dma_start(out=w, in_=io[nm][:, l])
                ws.append(w)
            return ws

        wtiles = load_qkvo(0)
        for l in range(n_layers):
            first = (l == 0)
            last = (l == n_layers - 1)

            wq_sb, wk_sb, wv_sb, wo_sb = wtiles

            kT = kqa_pool.tile([128, NJ, T_EXT], BF16, tag="kT")
            qT = kqa_pool.tile([128, NJ, T_OWN], BF16, tag="qT")
            v_sb = v_pool.tile([128, 8, H, HSP], FP8, tag="v")
            nc.vector.memset(v_sb[:, :, :, DH:HS], 1.0)

            def k_proj(tlo, thi):
                nn = thi - tlo
                for mj in range(NJ):
                    ps = ps_pool.tile([128, 512], F32, tag="big")
                    for kj in range(0, NJ, 2):
                        nc.tensor.matmul(
                            ps[:, 0:nn], wk_sb[:, kj:kj + 2, mj * 128:(mj + 1) * 128],
                            xn[:, kj:kj + 2, tlo:thi],
                            start=(kj == 0), stop=(kj == NJ - 2),
                            perf_mode=mybir.MatmulPerfMode.DoubleRow)
                    nc.vector.tensor_scalar(kT[:, mj, tlo:thi], ps[:, 0:nn],
                                            1.0 / FP8_WSCALE, bcol(col_bk(l, mj)),
                                            AluOpType.mult, AluOpType.add)

            def v_proj(tts):
                for tt in tts:
                    for ob in range(2):
                        psfull = ps_pool.tile([128, 512], F32, tag="big")
                        ps = psfull[:, 0:384]
                        for kj in range(0, NJ, 2):
                            nc.tensor.matmul(
                                ps, xn[:, kj:kj + 2, tt * 128:(tt + 1) * 128],
                                wv_sb[:, kj:kj + 2, ob * 384:(ob + 1) * 384],
                                start=(kj == 0), stop=False,
                                perf_mode=mybir.MatmulPerfMode.DoubleRow)
                        nc.tensor.matmul(
                            ps, ones_row_bf,
                            bv_sb[:, l * D + ob * 384: l * D + (ob + 1) * 384],
                            start=False, stop=True)
                        nc.vector.tensor_scalar(
                            v_sb[:, tt, ob * 6:(ob + 1) * 6, 0:DH],
                            ps.rearrange("p (h s) -> p h s", s=DH),
                            1.0 / (FP8_WSCALE * 8.0), None, AluOpType.mult)

            # -- own-token projections (independent of the halo AllGather) --
            with nc.named_scope(f"L{l}.qkv_own"):
                for mj in range(NJ):
                    ps = ps_pool.tile([128, 512], F32, tag="big")
                    for kj in range(0, NJ, 2):
                        nc.tensor.matmul(
                            ps, wq_sb[:, kj:kj + 2, mj * 128:(mj + 1) * 128],
                            xn[:, kj:kj + 2, 256:768],
                            start=(kj == 0), stop=(kj == NJ - 2),
                            perf_mode=mybir.MatmulPerfMode.DoubleRow)
                    nc.vector.tensor_scalar(qT[:, mj, :], ps,
                                            1.0 / FP8_WSCALE, bcol(col_bq(l, mj)),
                                            AluOpType.mult, AluOpType.add)
                if first:
                    k_proj(0, 512)
                    k_proj(512, 1024)
                    v_proj(range(8))
                else:
                    k_proj(256, 768)
                    v_proj(range(2, 6))

            # -- attention (heads software-pipelined) --
            attnT = kqa_pool.tile([128, NJ, T_OWN], FP8, tag="attnT")

            def emit_scores(n, h):
                jh, po = h // 2, (h % 2) * 64
                ems = []
                for t in range(3):
                    ps = ps_pool.tile([128, 512], F32, tag="big")
                    for half in range(2):
                        kofs = n * 256 + (2 * t + half) * 128
                        nc.tensor.matmul(
                            ps[:, half * 256:(half + 1) * 256],
                            kT[po:po + 64, jh, kofs:kofs + 128],
                            qT[po:po + 64, jh, n * 256:(n + 1) * 256],
                            start=True, stop=True)
                    e = exp_pool.tile([128, 512], FP8, tag="exp")
                    nc.scalar.activation(e, ps, AF.Exp)
                    if t == 1 and not need_mid:
                        ems.append(e)
                    else:
                        em = exp_pool.tile([128, 512], FP8, tag="em")
                        nc.vector.tensor_tensor(
                            em, e, mask_sb[:, n, mslot[t], :], AluOpType.mult)
                        ems.append(em)
                return ems

            dens = [den_pool.tile([1, H * 256], BF16, tag="den", name=f"den{i}")
                    for i in range(2)]

            def emit_av(n, h, ems):
                jh, po = h // 2, (h % 2) * 64
                aps = attn_ps.tile([HS, 256], F32, tag="attn")
                for t in range(3):
                    tt = n * 2 + 2 * t
                    nc.tensor.matmul(
                        aps, v_sb[:, tt:tt + 2, h, 0:HS],
                        ems[t].rearrange("p (k q) -> p k q", k=2),
                        start=(t == 0), stop=(t == 2),
                        perf_mode=mybir.MatmulPerfMode.DoubleRow)
                nc.vector.tensor_copy(dens[n][:, h * 256:(h + 1) * 256],
                                      aps[64:65, :])
                nc.vector.tensor_copy(
                    attnT[po:po + 64, jh, n * 256:(n + 1) * 256], aps[0:64, :])

            def emit_norm(n):
                # grouped reciprocals: one ACT table load for the whole block
                for h in range(H):
                    jh, po = h // 2, (h % 2) * 64
                    rec = vec_pool.tile([1, 256], F32, tag="rec")
                    _act_raw(nc, rec, dens[n][:, h * 256:(h + 1) * 256],
                             AF.Reciprocal)
                    bc = attn_ps.tile([64, 256], F32, tag="attn")
                    nc.tensor.matmul(bc, row64[0:1, 0:64], rec,
                                     start=True, stop=True)
                    sl = attnT[po:po + 64, jh, n * 256:(n + 1) * 256]
                    nc.vector.tensor_tensor(sl, sl, bc, AluOpType.mult)

            pend = None
            for n in range(2):
                if n == 1:
                    if not first:
                        with nc.named_scope(f"L{l}.halo_right"):
                            k_proj(768, 1024)
                            v_proj(range(6, 8))
                    if not last:
                        wtiles = load_qkvo(l + 1)
                with nc.named_scope(f"L{l}.attn{n}"):
                    if n == 0 and not first:
                        with nc.named_scope(f"L{l}.halo_left"):
                            k_proj(0, 256)
                            v_proj(range(0, 2))
                    for h in range(H):
                        ems = emit_scores(n, h)
                        if pend is not None:
                            emit_av(*pend)
                        pend = (n, h, ems)
                        if n == 1 and h == 1:
                            emit_norm(0)
            emit_av(*pend)
            emit_norm(1)

            # -- Wo projection + residual -> r1 --
            r1 = r_pool.tile([128, NJ, T_OWN], BF16, tag="r")
            with nc.named_scope(f"L{l}.wo"):
                for mj in range(NJ):
                    ps = ps_pool.tile([128, 512], F32, tag="big")
                    for kj in range(0, NJ, 2):
                        nc.tensor.matmul(
                            ps, wo_sb[:, kj:kj + 2, mj * 128:(mj + 1) * 128],
                            attnT[:, kj:kj + 2, :],
                            start=(kj == 0), stop=(kj == NJ - 2),
                            perf_mode=mybir.MatmulPerfMode.DoubleRow)
                    t = sq_pool.tile([128, 512], F32, tag="ao", bufs=2)
                    nc.scalar.activation(t, ps, AF.Identity,
                                         bias=bcol(col_bo(l, mj)),
                                         scale=1.0 / FP8_WSCALE)
                    nc.vector.tensor_tensor(r1[:, mj, :], t, r0[:, mj, :],
                                            AluOpType.add)

            # -- LN1 -> xn1b (bf16; also the FFN residual) --
            xn1b = xb_pool.tile([128, NJ, T_OWN], BF16, tag="xn1b")
            xn1b8 = xb_pool.tile([128, NJ, T_OWN], FP8, tag="xn1b8")
            with nc.named_scope(f"L{l}.ln1"):
                layer_norm(
                    lambda j, blk: r1[:, j, :], lambda j, blk: r1[:, j, :], 1,
                    lambda j: col_ln1g(l, j), lambda j: col_ln1b(l, j),
                    lambda j, blk: [(xn1b[:, j, :], 0, 512),
                                    (xn1b8[:, j, :], 0, 512)],
                    ones_st=ones_col_bf)

            # -- FFN --
            r2 = r_pool.tile([128, NJ, T_OWN], BF16, tag="r")
            with nc.named_scope(f"L{l}.ffn"):
                f2ps = [ps_pool.tile([128, 512], F32, tag="big",
                                     name=f"f2ps{mj}") for mj in range(NJ)]
                for q in range(4):
                    wf1_sb = wf1_pool.tile([128, NJ, D], FP8, tag="wf1")
                    nc.gpsimd.dma_start(out=wf1_sb, in_=io["wf1"][:, l, q])
                    wf2_sb = wf2_pool.tile([128, NJ, D], BF16, tag="wf2")
                    nc.gpsimd.dma_start(out=wf2_sb, in_=io["wf2"][:, l, q])
                    hq = h_pool.tile([128, NJ, T_OWN], BF16, tag="h", bufs=2)
                    for mj6 in range(NJ):
                        ps = attn_ps.tile([128, 512], F32, tag="attn")
                        for kj in range(0, NJ, 2):
                            nc.tensor.matmul(
                                ps, wf1_sb[:, kj:kj + 2, mj6 * 128:(mj6 + 1) * 128],
                                xn1b8[:, kj:kj + 2, :],
                                start=(kj == 0), stop=(kj == NJ - 2),
                                perf_mode=mybir.MatmulPerfMode.DoubleRow)
                        nc.scalar.activation(
                            hq[:, mj6, :], ps, AF.Gelu,
                            bias=bcol(col_bff1(l, q * NJ + mj6)),
                            scale=1.0 / FP8_WSCALE)
                    for mj in range(NJ):
                        for kj in range(NJ):
                            nc.tensor.matmul(
                                f2ps[mj], wf2_sb[:, kj, mj * 128:(mj + 1) * 128],
                                hq[:, kj, :],
                                start=(q == 0 and kj == 0),
                                stop=(q == 3 and kj == NJ - 1))
                for mj in range(NJ):
                    t = sq_pool.tile([128, 512], F32, tag="ao", bufs=2)
                    nc.scalar.activation(t, f2ps[mj], AF.Identity,
                                         bias=bcol(col_bff2(l, mj)))
                    nc.vector.tensor_tensor(r2[:, mj, :], t, xn1b[:, mj, :],
                                            AluOpType.add)

            # -- LN2 -> next xn (+ next r0), in halves; right half first so
            # its AllGather (which feeds the left halos) launches early --
            xn_next = None if last else xn_pool.tile([128, NJ, T_EXT], FP8, tag="xn")
            xn2f = r_pool.tile([128, NJ, T_OWN], BF16, tag="r")
            HLO = (256, 0)  # blk0 = own tokens 256:512, blk1 = 0:256

            def ln2_outs(j, blk, xn_next=xn_next, xn2f=xn2f, last=last):
                lo = HLO[blk]
                dsts = [(xn2f[:, j, lo:lo + 256], 0, 256)]
                if not last:
                    dsts.append((xn_next[:, j, 256 + lo:512 + lo], 0, 256))
                return dsts

            def ln2_blk_done(blk, xn_next=xn_next, last=last, l=l):
                if last:
                    return
                with nc.named_scope(f"L{l}.allgather{blk}"):
                    agi = dram_pool.tile([D, 256], FP8, tag="agi")
                    ago = dram_pool.tile([P * D, 256], FP8, tag="ago",
                                         addr_space="Shared")
                    lo = HLO[blk]
                    nc.sync.dma_start(
                        out=agi.rearrange("(j p) t -> p j t", p=128),
                        in_=xn_next[:, :, 256 + lo:512 + lo])
                    nc.gpsimd.collective_compute(
                        "AllGather", AluOpType.bypass,
                        replica_groups=[list(range(P))],
                        ins=[agi.opt()], outs=[ago.opt()])
                    agv = ago.rearrange("(r j p) t -> p (r j) t", j=NJ, p=128)
                    if blk == 0:
                        nc.sync.dma_start(out=xn_next[:, :, 0:256],
                                          in_=agv[:, bass.ds(lidx6, NJ), :])
                    else:
                        nc.sync.dma_start(out=xn_next[:, :, 768:1024],
                                          in_=agv[:, bass.ds(ridx6, NJ), :])

            with nc.named_scope(f"L{l}.ln2"):
                layer_norm(
                    lambda j, blk, r2=r2: r2[:, j, HLO[blk]:HLO[blk] + 256],
                    lambda j, blk, r2=r2: r2[:, j, HLO[blk]:HLO[blk] + 256], 2,
                    lambda j: col_ln2g(l, j), lambda j: col_ln2b(l, j),
                    ln2_outs, ones_st=ones_col_bf, bw=256,
                    blk_done=ln2_blk_done)

            if not last:
                xn = xn_next
            r0 = xn2f

        # ---------------- pooling partials ----------------
        with nc.named_scope("pool"):
            accs = acc_pool.tile([128, NJ], F32, tag="accs")
            if need_mid:
                mb = ps_pool.tile([128, 512], F32, tag="big")
                nc.tensor.matmul(mb, ones_row, maskf_sb, start=True, stop=True)
                for j in range(NJ):
                    mskd = tmp_pool.tile([128, 512], F32, tag="tmp")
                    nc.vector.tensor_tensor(mskd, r0[:, j, :], mb, AluOpType.mult)
                    scr = sq_pool.tile([128, 512], F32, tag="sq", bufs=2)
                    nc.scalar.activation(scr, mskd, AF.Copy,
                                         accum_out=accs[:, j:j + 1])
            else:
                for j in range(NJ):
                    scr = sq_pool.tile([128, 512], F32, tag="sq", bufs=2)
                    nc.scalar.activation(scr, r0[:, j, :], AF.Copy,
                                         accum_out=accs[:, j:j + 1])
            nc.sync.dma_start(out=io["pool_out"], in_=accs)


# --------------------------------------------------------------------------
# host side
# --------------------------------------------------------------------------

def _build_masks(attention_mask):
    """[P, 2, 3*C, C] multiplicative float mask (band + attn mask + edges)."""
    maskf = np.asarray(attention_mask, np.float32).reshape(S)
    masks = np.zeros((P, 2, 3 * C, C), np.float32)
    qi = np.arange(C)[None, :]
    kj = np.arange(3 * C)[:, None]
    band = (np.abs(kj - C - qi) <= W)
    for c in range(P):
        for n in range(2):
            g0 = c * T_OWN + n * C
            kg = g0 - C + np.arange(3 * C)
            valid = (kg >= 0) & (kg < S)
            mvals = np.where(valid, maskf[np.clip(kg, 0, S - 1)], 0.0)
            masks[c, n] = band * (mvals[:, None] > 0)
    return masks


_cache = {}


def kernel(input_ids, attention_mask, word_emb, pos_emb, emb_g, emb_b,
           Wq, Wk, Wv, Wo, bq, bk, bv, bo, ln1_g, ln1_b,
           Wff1, bff1, Wff2, bff2, ln2_g, ln2_b,
           W1, b1, W2, b2, W3, b3):
    to32 = lambda a: np.ascontiguousarray(np.asarray(a, np.float32))
    tob = lambda a: np.asarray(a, np.float32).astype(NPBF16)
    to8 = lambda a: np.clip(np.asarray(a, np.float32) * FP8_WSCALE,
                            -240, 240).astype(ml_dtypes.float8_e4m3)
    ids = np.asarray(input_ids).reshape(S)
    word_emb, pos_emb = to32(word_emb), to32(pos_emb)
    emb = word_emb[ids] + pos_emb                      # [S, D] host gather
    masks = _build_masks(attention_mask)
    maskf = np.asarray(attention_mask, np.float32).reshape(S)
    need_mid = not bool(np.asarray(attention_mask).all())

    scale = 1.0 / np.sqrt(np.float32(DH))
    wq_s = to32(Wq) * scale
    bq_s = to32(bq) * scale

    bias_cols = np.zeros((128, NB), np.float32)
    for j in range(NJ):
        sl = slice(j * 128, (j + 1) * 128)
        bias_cols[:, col_emb_g(j)] = to32(emb_g)[sl]
        bias_cols[:, col_emb_b(j)] = to32(emb_b)[sl]
    for l in range(L):
        for j in range(NJ):
            sl = slice(j * 128, (j + 1) * 128)
            bias_cols[:, col_bq(l, j)] = bq_s[l][sl]
            bias_cols[:, col_bk(l, j)] = to32(bk)[l][sl]
            bias_cols[:, col_bo(l, j)] = to32(bo)[l][sl]
            bias_cols[:, col_bff2(l, j)] = to32(bff2)[l][sl]
            bias_cols[:, col_ln1g(l, j)] = to32(ln1_g)[l][sl]
            bias_cols[:, col_ln1b(l, j)] = to32(ln1_b)[l][sl]
            bias_cols[:, col_ln2g(l, j)] = to32(ln2_g)[l][sl]
            bias_cols[:, col_ln2b(l, j)] = to32(ln2_b)[l][sl]
        for j in range(NJF):
            bias_cols[:, col_bff1(l, j)] = to32(bff1)[l][j * 128:(j + 1) * 128]

    # weights pre-transposed host-side to [128, L, (q,) kj, out] so each
    # per-layer DMA reads one contiguous run per partition
    wq_b = np.ascontiguousarray(
        to8(wq_s).reshape(L, NJ, 128, D).transpose(2, 0, 1, 3))
    wk_b = np.ascontiguousarray(
        to8(Wk).reshape(L, NJ, 128, D).transpose(2, 0, 1, 3))
    wv_b = np.ascontiguousarray(
        to8(Wv).reshape(L, NJ, 128, D).transpose(2, 0, 1, 3))
    wo_b = np.ascontiguousarray(
        to8(Wo).reshape(L, NJ, 128, D).transpose(2, 0, 1, 3))
    wf1_b = np.ascontiguousarray(
        to8(Wff1).reshape(L, NJ, 128, 4, D).transpose(2, 0, 3, 1, 4))
    wf2_b = np.ascontiguousarray(
        tob(Wff2).reshape(L, 4, NJ, 128, D).transpose(3, 0, 1, 2, 4))
    bv_b = np.ascontiguousarray(tob(np.asarray(bv, np.float32)
                                    * FP8_WSCALE).reshape(1, L * D))

    n_layers = int(os.environ.get("KERNEL_LAYERS", L))
    key = (n_layers, need_mid)
    if key not in _cache:
        _cache[key] = build_program(n_layers, need_mid)
    nc = _cache[key]

    in_maps = []
    for c in range(P):
        lo, hi = c * T_OWN - C, c * T_OWN + T_OWN + C
        e = np.zeros((T_EXT, D), np.float32)
        s0, s1 = max(lo, 0), min(hi, S)
        e[s0 - lo:s1 - lo] = emb[s0:s1]
        mp = np.zeros((2, 3, 128, 512), np.float32)
        for n in range(2):
            for t in range(3):
                for half in range(2):
                    mp[n, t, :, half * 256:(half + 1) * 256] = \
                        masks[c, n][(2 * t + half) * 128:(2 * t + half + 1) * 128, :]
        in_maps.append({
            "embT": np.ascontiguousarray(e.T.astype(NPBF16)),
            "wq": wq_b, "wk": wk_b, "wv": wv_b, "wo": wo_b,
            "wf1": wf1_b, "wf2": wf2_b,
            "bias_cols": bias_cols,
            "bv_rows": bv_b,
            "maskT": np.ascontiguousarray(
                mp.transpose(2, 0, 1, 3)[:, :, ([0, 1, 2] if need_mid else [0, 2])]
                .astype(NPBF16)),
            "maskf": np.ascontiguousarray(
                maskf[c * T_OWN:(c + 1) * T_OWN].reshape(1, T_OWN)),
        })

    trace = os.environ.get("KERNEL_TRACE", "0") == "1"
    if trace:
        _install_ntff_hook()
    res = run_bass_kernel_spmd(nc, in_maps, core_ids=list(range(P)), trace=trace)
    kernel.last_exec_time_ns = res.exec_time_ns
    kernel.last_results = res.results
    kernel.last_res = res

    pooled = np.zeros(D, np.float64)
    for c in range(P):
        po = np.asarray(res.results[c]["pool_out"], np.float64)   # [128, NJ]
        pooled += po.T.reshape(D)                                 # f = j*128+p
    msum = max(maskf.sum(), 1e-9)
    pooled = (pooled / msum).astype(np.float32)

    h1 = np.maximum(pooled @ to32(W1) + to32(b1), 0)
    h2 = np.maximum(h1 @ to32(W2) + to32(b2), 0)
    pred = (h2 @ to32(W3) + to32(b3))[None].astype(np.float32)
    return pred, pred


kernel.last_exec_time_ns = None
kernel.last_results = None
kernel.last_res = None
